# revision 1
# baseline (speedup 1.0000x reference)
"""CardEncoder Trainium2 kernel.

Model (per sequence of L=16 tokens): embed(32) -> bidirectional LSTM(32) ->
concat final states -> per-batch dense (4096 -> 64) -> tanh.

Strategy (pure data parallel, 8 cores, batch-sharded):
  * Host packs an augmented gather table [10112, 128] bf16 per vocab row:
      [ h_fw slot (zeros 0:32) | embedding (32:64) | 1.0 bias (64) |
        h_bw slot (zeros 65:97) | zeros ]
  * Device gathers rows with dma_gather(transpose=True) -> feature-major
    tiles G[128, T*NT] (columns = (t, seq)); the constant-1 row turns the
    LSTM bias into a matmul row; the zero h-slots are overwritten in-place
    with the running hidden state so each step is ONE K=65 matmul per gate.
  * 4 streams (2 seq-tiles x {fw, bw}) stacked on PSUM partition quarters via
    tile_position col-groups, so all elementwise work runs on full
    [128, NT] tiles.
  * LSTM cell: i,f,o = sigmoid, g = tanh (ACT); c = f*c + i*g, h = o*tanh(c)
    (DVE); h copied into the next step's h-slot of G.
  * Dense head on device; host transposes [64, B] -> [B, 64].
  * mask_zero=True handling: token==0 steps must leave (h, c) unchanged.
    Zero tokens occur w.p. 1e-4; the device ignores masking and the host
    recomputes the ~0.02% of affected sequences exactly and patches the
    affected output rows.
"""

import os
import numpy as np
import ml_dtypes

os.environ.setdefault("JAX_PLATFORMS", "cpu")

import concourse.bass as bass
import concourse.bacc as bacc
import concourse.mybir as mybir
import concourse.tile as tile
from concourse import bass_utils

BF16 = ml_dtypes.bfloat16

B, P, L = 2048, 64, 16
H = 32                      # LSTM units per direction
VOC = 10000
VOCP = 10112                # padded to 79 * 128
N_CORES = 8
B_LOC = B // N_CORES        # 256 batches per core
NSEQ = B_LOC * P            # 16384 sequences per core
NT = 512                    # sequences per tile
T = L

# G tile partition layout (SBUF APs must start at partition 0/32/64/96):
#   [ h_fw slot 0:32 | emb 32:64 | emb copy 64:96 | h_bw slot 96:128 ]
# fw rhs = rows 0:64 [h_fw, emb];  bw rhs = rows 64:128 [emb, h_bw].
HFW0 = 0
EMB0 = 32
EMB1 = 64
HBW0 = 96
KDIM = 64                   # matmul contraction size


def _f32(x):
    return np.asarray(x, np.float32)


# ---------------------------------------------------------------------------
# device kernel
# ---------------------------------------------------------------------------

def build_kernel(nseq=NSEQ, mode="full"):
    ntiles = nseq // NT
    npairs = ntiles // 2
    nbatch = nseq // P

    nc = bacc.Bacc("TRN2", target_bir_lowering=False, debug=False,
                   enable_asserts=False, num_devices=N_CORES)

    table_d = nc.dram_tensor("table", [VOCP, 128], mybir.dt.bfloat16,
                             kind="ExternalInput")
    idx_d = nc.dram_tensor("idx", [128, ntiles * NT * T // 16], mybir.dt.int16,
                           kind="ExternalInput")
    wf_d = nc.dram_tensor("wf", [KDIM, 128], mybir.dt.bfloat16,
                          kind="ExternalInput")
    wb_d = nc.dram_tensor("wb", [128, 128], mybir.dt.bfloat16,
                          kind="ExternalInput")
    bv_d = nc.dram_tensor("bv", [128, 4], mybir.dt.float32,
                          kind="ExternalInput")
    wd_d = nc.dram_tensor("wd", [64, 4096], mybir.dt.bfloat16,
                          kind="ExternalInput")
    bd_d = nc.dram_tensor("bd", [64, 1], mybir.dt.float32,
                          kind="ExternalInput")
    out_d = nc.dram_tensor("out", [64, nbatch], mybir.dt.float32,
                           kind="ExternalOutput")
    state_d = nc.dram_tensor("state", [64, nseq], mybir.dt.bfloat16,
                             kind="ExternalOutput")

    FP32 = mybir.dt.float32
    BF = mybir.dt.bfloat16
    SIG = mybir.ActivationFunctionType.Sigmoid
    TANH = mybir.ActivationFunctionType.Tanh

    with tile.TileContext(nc) as tc:
        with tc.tile_pool(name="const", bufs=1) as cpool:
          with tc.tile_pool(name="gbuf", bufs=2) as gpool, \
               tc.tile_pool(name="work", bufs=2) as wpool, \
               tc.tile_pool(name="zps", bufs=2, space="PSUM") as zpool:

            wf = cpool.tile([KDIM, 128], BF)
            nc.sync.dma_start(out=wf[:, :], in_=wf_d.ap())
            # bw weights live at partition base 64: walrus requires matmul
            # fmap and weight to share the same SB start partition, and the
            # bw rhs is G[64:128]. Host pads to [128, 128] (top half zeros)
            # so the DMA itself writes at partition base 0.
            wb_t = cpool.tile([128, 128], BF)
            nc.sync.dma_start(out=wb_t[:, :], in_=wb_d.ap())
            wb = wb_t[64:128, :]
            bv = cpool.tile([128, 4], FP32)
            nc.sync.dma_start(out=bv[:, :], in_=bv_d.ap())
            idx_sb = cpool.tile([128, ntiles * NT * T // 16], mybir.dt.int16)
            nc.sync.dma_start(out=idx_sb[:, :], in_=idx_d.ap())
            state = cpool.tile([64, nseq], BF)

            IDXW = NT * T // 16      # idx columns per tile

            def gather_tile(j):
                g = gpool.tile([128, 1, T * NT], BF, tag=f"g{j % 2}")
                nc.gpsimd.dma_gather(
                    out_ap=g[:, :, :],
                    in_ap=table_d.ap(),
                    idxs_ap=idx_sb[:, j * IDXW:(j + 1) * IDXW],
                    num_idxs=T * NT,
                    num_idxs_reg=T * NT,
                    elem_size=128,
                    transpose=True,
                    single_packet=False,
                )
                return g

            for pair in range(npairs):
                if mode == "empty":
                    break
                if mode == "compute":
                    ga = gpool.tile([128, 1, T * NT], BF, tag="g0",
                                    name=f"ga{pair}")
                    gb = gpool.tile([128, 1, T * NT], BF, tag="g1",
                                    name=f"gb{pair}")
                else:
                    ga = gather_tile(2 * pair)
                    gb = gather_tile(2 * pair + 1)
                gs = [ga, gb]
                if mode == "gather":
                    # consume G so nothing gets dead-code-eliminated
                    for gi2, g_t in enumerate(gs):
                        col0 = (2 * pair + gi2) * NT
                        nc.vector.tensor_copy(
                            state[0:32, col0:col0 + NT],
                            g_t[32:64, 0, (T - 1) * NT:T * NT])
                    continue
                c_all = wpool.tile([128, NT], FP32, tag="c")

                for tau in range(T):
                    # one PSUM bank per gate, stream s on partition quarter s
                    zt = [zpool.tile([128, NT], FP32, tag=f"z{gi}",
                                     name=f"z{gi}_{pair}_{tau}")
                          for gi in range(4)]
                    for s in range(4):
                        g_t = gs[s // 2]
                        bw = s % 2
                        blk = (T - 1 - tau) if bw else tau
                        lo = EMB1 if bw else HFW0
                        w_s = wb if bw else wf
                        rhs = g_t[lo:lo + KDIM, 0, blk * NT:(blk + 1) * NT]
                        for gi in range(4):   # gate order i,f,g,o
                            nc.tensor.matmul(
                                zt[gi][32 * s:32 * s + 32, :],
                                w_s[:, 32 * gi:32 * gi + 32], rhs,
                                start=True, stop=True,
                                tile_position=(64 if bw else 0, 32 * s))

                    ui = wpool.tile([128, NT], BF, tag="ui")
                    nc.scalar.activation(ui[:, :], zt[0][:, :], SIG,
                                         bias=bv[:, 0:1])
                    uf = wpool.tile([128, NT], BF, tag="uf")
                    nc.scalar.activation(uf[:, :], zt[1][:, :], SIG,
                                         bias=bv[:, 1:2])
                    g_all = wpool.tile([128, NT], BF, tag="gall")
                    nc.scalar.activation(g_all[:, :], zt[2][:, :], TANH,
                                         bias=bv[:, 2:3])
                    uo = wpool.tile([128, NT], BF, tag="uo")
                    nc.scalar.activation(uo[:, :], zt[3][:, :], SIG,
                                         bias=bv[:, 3:4])
                    ui, uf, uo = ui[:, :], uf[:, :], uo[:, :]

                    if tau == 0:
                        # c = i * g  (h-slots and previous c are zero)
                        nc.vector.tensor_mul(c_all[:, :], ui, g_all[:, :])
                    else:
                        t1 = wpool.tile([128, NT], BF, tag="t1")
                        nc.vector.tensor_mul(t1[:, :], ui, g_all[:, :])
                        t2 = wpool.tile([128, NT], FP32, tag="t2")
                        nc.vector.tensor_mul(t2[:, :], uf, c_all[:, :])
                        nc.vector.tensor_add(c_all[:, :], t1[:, :], t2[:, :])

                    tc_t = wpool.tile([128, NT], BF, tag="tc")
                    nc.scalar.activation(tc_t[:, :], c_all[:, :], TANH)
                    h_new = wpool.tile([128, NT], BF, tag="hn")
                    nc.vector.tensor_mul(h_new[:, :], uo, tc_t[:, :])

                    for s in range(4):
                        g_t = gs[s // 2]
                        bw = s % 2
                        src = h_new[32 * s:32 * s + 32, :]
                        if tau == T - 1:
                            col0 = (2 * pair + s // 2) * NT
                            dst = state[32 * bw:32 * bw + 32,
                                        col0:col0 + NT]
                        else:
                            nblk = (T - 2 - tau) if bw else (tau + 1)
                            h0 = HBW0 if bw else HFW0
                            dst = g_t[h0:h0 + 32, 0,
                                      nblk * NT:(nblk + 1) * NT]
                        nc.vector.tensor_copy(dst, src)

          # ---- dense head: out[c, b] = tanh(sum_p Wd_p.T @ state_p + bd)
          if True:
            wd = cpool.tile([64, 4096], BF)
            nc.sync.dma_start(out=wd[:, :], in_=wd_d.ap())
            bd = cpool.tile([64, 1], FP32)
            nc.sync.dma_start(out=bd[:, :], in_=bd_d.ap())

            with tc.tile_pool(name="head", bufs=1, space="PSUM") as hpool:
                st_r = state[:, :].rearrange("p (b q) -> p q b", q=P)
                hp = hpool.tile([64, nbatch], FP32)
                for p in range(P):
                    nc.tensor.matmul(hp[:, :], wd[:, 64 * p:64 * p + 64],
                                     st_r[:, p:p + 1, :],
                                     start=(p == 0), stop=(p == P - 1))
                out_sb = cpool.tile([64, nbatch], FP32)
                nc.scalar.activation(out_sb[:, :], hp[:, :], TANH,
                                     bias=bd[:, :])
                nc.sync.dma_start(out=out_d.ap(), in_=out_sb[:, :])
                nc.sync.dma_start(out=state_d.ap(), in_=state[:, :])

    nc.compile()
    return nc


# ---------------------------------------------------------------------------
# host-side packing
# ---------------------------------------------------------------------------

def pack_table(embed_table):
    tbl = np.zeros((VOCP, 128), np.float32)
    tbl[:VOC, EMB0:EMB0 + 32] = _f32(embed_table)
    tbl[:VOC, EMB1:EMB1 + 32] = _f32(embed_table)
    return tbl.astype(BF16)


def pack_idx(x_core, nseq=NSEQ):
    """x_core: [nseq, T] int32 -> wrapped int16 [128, ntiles * T*NT/16]."""
    ntiles = nseq // NT
    cols = []
    for j in range(ntiles):
        u = x_core[j * NT:(j + 1) * NT, :].T.reshape(-1)     # t-major [T*NT]
        w = u.reshape(-1, 16).T                               # [16, T*NT/16]
        cols.append(np.tile(w, (8, 1)))
    return np.concatenate(cols, axis=1).astype(np.int16)


def pack_weights(Wk, Wr, b):
    Wk, Wr, b = _f32(Wk), _f32(Wr), _f32(b)
    wf = np.concatenate([Wr, Wk], 0)                          # [64, 128]
    wb = np.concatenate([np.zeros((64, 128), np.float32), Wk, Wr], 0)
    bv = np.tile(b.reshape(4, 32), (1, 4)).reshape(4, 128).T  # [128, 4]
    return wf.astype(BF16), wb.astype(BF16), np.ascontiguousarray(bv, np.float32)


def pack_wd(Wd):
    w = _f32(Wd).reshape(P, 64, 64).transpose(1, 0, 2).reshape(64, 4096)
    return w.astype(BF16)


# ---------------------------------------------------------------------------
# host reference bits for the zero-token fixup
# ---------------------------------------------------------------------------

def _np_lstm_last_h(emb, mask, Wk, Wr, b):
    n = emb.shape[0]
    h = np.zeros((n, H), np.float32)
    c = np.zeros((n, H), np.float32)
    for t in range(emb.shape[1]):
        z = emb[:, t, :] @ Wk + h @ Wr + b
        i = 1.0 / (1.0 + np.exp(-z[:, 0:32]))
        f = 1.0 / (1.0 + np.exp(-z[:, 32:64]))
        g = np.tanh(z[:, 64:96])
        o = 1.0 / (1.0 + np.exp(-z[:, 96:128]))
        c_new = f * c + i * g
        h_new = o * np.tanh(c_new)
        m = mask[:, t][:, None]
        h = np.where(m, h_new, h)
        c = np.where(m, c_new, c)
    return h


def _host_fixup(out, state_all, x_flat, embed_table, Wk, Wr, b, Wd, bd):
    """Recompute rows whose sequences contain a zero token."""
    mask = x_flat != 0
    bad_seq = np.nonzero(~mask.all(axis=1))[0]
    if bad_seq.size == 0:
        return out
    emb = _f32(embed_table)[x_flat[bad_seq]]
    h_fw = _np_lstm_last_h(emb, mask[bad_seq], _f32(Wk), _f32(Wr), _f32(b))
    h_bw = _np_lstm_last_h(emb[:, ::-1, :], mask[bad_seq][:, ::-1],
                           _f32(Wk), _f32(Wr), _f32(b))
    state_all = state_all.copy()
    state_all[bad_seq] = np.concatenate([h_fw, h_bw], axis=1)
    bad_rows = np.unique(bad_seq // P)
    st = state_all[bad_rows[:, None] * P + np.arange(P)[None, :]]
    st = st.reshape(bad_rows.size, P * 64)
    out[bad_rows] = np.tanh(st @ _f32(Wd) + _f32(bd))
    return out


# ---------------------------------------------------------------------------
# entry point
# ---------------------------------------------------------------------------

_NC_CACHE = {}


def _get_nc(mode="full"):
    key = "nc" + mode
    if key not in _NC_CACHE:
        _NC_CACHE[key] = build_kernel(mode=mode)
    return _NC_CACHE[key]


def run_device(inputs, trace=False):
    x = np.asarray(inputs["x"])
    table = pack_table(inputs["embed_table"])
    wf, wb, bv = pack_weights(inputs["Wk"], inputs["Wr"], inputs["b"])
    wd = pack_wd(inputs["Wd"])
    bd = _f32(inputs["bd"]).reshape(64, 1)

    x_flat = x.reshape(B * P, L)
    in_maps = []
    for k in range(N_CORES):
        x_core = x_flat[k * NSEQ:(k + 1) * NSEQ]
        in_maps.append({
            "table": table,
            "idx": pack_idx(x_core),
            "wf": wf,
            "wb": wb,
            "bv": bv,
            "wd": wd,
            "bd": bd,
        })

    nc = _get_nc()
    res = bass_utils.run_bass_kernel_spmd(
        nc, in_maps, core_ids=list(range(N_CORES)), trace=trace)

    out = np.empty((B, 64), np.float32)
    state_all = np.empty((B * P, 64), np.float32)
    for k in range(N_CORES):
        out[k * B_LOC:(k + 1) * B_LOC] = res.results[k]["out"].T
        state_all[k * NSEQ:(k + 1) * NSEQ] = \
            _f32(res.results[k]["state"]).T
    return out, state_all, res


def kernel(x, embed_table, Wk, Wr, b, Wd, bd):
    inputs = dict(x=x, embed_table=embed_table, Wk=Wk, Wr=Wr, b=b,
                  Wd=Wd, bd=bd)
    out, state_all, _ = run_device(inputs)
    out = _host_fixup(out, state_all, np.asarray(x).reshape(B * P, L),
                      embed_table, Wk, Wr, b, Wd, bd)
    return out



# revision 6
# speedup vs baseline: 1.1007x; 1.1007x over previous
"""CardEncoder Trainium2 kernel.

Model (per sequence of L=16 tokens): embed(32) -> bidirectional LSTM(32) ->
concat final states -> per-batch dense (4096 -> 64) -> tanh.

Strategy (pure data parallel, 8 cores, batch-sharded):
  * Host packs an augmented gather table [10112, 128] bf16 per vocab row:
      [ h_fw slot (zeros 0:32) | embedding (32:64) | 1.0 bias (64) |
        h_bw slot (zeros 65:97) | zeros ]
  * Device gathers rows with dma_gather(transpose=True) -> feature-major
    tiles G[128, T*NT] (columns = (t, seq)); the constant-1 row turns the
    LSTM bias into a matmul row; the zero h-slots are overwritten in-place
    with the running hidden state so each step is ONE K=65 matmul per gate.
  * 4 streams (2 seq-tiles x {fw, bw}) stacked on PSUM partition quarters via
    tile_position col-groups, so all elementwise work runs on full
    [128, NT] tiles.
  * LSTM cell: i,f,o = sigmoid, g = tanh (ACT); c = f*c + i*g, h = o*tanh(c)
    (DVE); h copied into the next step's h-slot of G.
  * Dense head on device; host transposes [64, B] -> [B, 64].
  * mask_zero=True handling: token==0 steps must leave (h, c) unchanged.
    Zero tokens occur w.p. 1e-4; the device ignores masking and the host
    recomputes the ~0.02% of affected sequences exactly and patches the
    affected output rows.
"""

import os
import numpy as np
import ml_dtypes

os.environ.setdefault("JAX_PLATFORMS", "cpu")

import concourse.bass as bass
import concourse.bacc as bacc
import concourse.mybir as mybir
import concourse.tile as tile
from concourse import bass_utils

BF16 = ml_dtypes.bfloat16

B, P, L = 2048, 64, 16
H = 32                      # LSTM units per direction
VOC = 10000
VOCP = 10112                # padded to 79 * 128
N_CORES = 8
B_LOC = B // N_CORES        # 256 batches per core
NSEQ = B_LOC * P            # 16384 sequences per core
NT = 512                    # sequences per tile
T = L

# G tile partition layout (SBUF APs must start at partition 0/32/64/96):
#   [ h_fw slot 0:32 | emb 32:64 | emb copy 64:96 | h_bw slot 96:128 ]
# fw rhs = rows 0:64 [h_fw, emb];  bw rhs = rows 64:128 [emb, h_bw].
HFW0 = 0
EMB0 = 32
EMB1 = 64
HBW0 = 96
KDIM = 64                   # matmul contraction size


def _f32(x):
    return np.asarray(x, np.float32)


# ---------------------------------------------------------------------------
# device kernel
# ---------------------------------------------------------------------------

def build_kernel(nseq=NSEQ, mode="full", hw_loop=False):
    ntiles = nseq // NT
    npairs = ntiles // 2
    nbatch = nseq // P

    nc = bacc.Bacc("TRN2", target_bir_lowering=False, debug=False,
                   enable_asserts=False, num_devices=N_CORES)

    table_d = nc.dram_tensor("table", [VOCP, 128], mybir.dt.bfloat16,
                             kind="ExternalInput")
    idx_d = nc.dram_tensor("idx", [128, ntiles * NT * T // 16], mybir.dt.int16,
                           kind="ExternalInput")
    wf_d = nc.dram_tensor("wf", [KDIM, 128], mybir.dt.bfloat16,
                          kind="ExternalInput")
    wb_d = nc.dram_tensor("wb", [128, 128], mybir.dt.bfloat16,
                          kind="ExternalInput")
    bv_d = nc.dram_tensor("bv", [128, 4], mybir.dt.float32,
                          kind="ExternalInput")
    wd_d = nc.dram_tensor("wd", [64, 4096], mybir.dt.bfloat16,
                          kind="ExternalInput")
    bd_d = nc.dram_tensor("bd", [64, 1], mybir.dt.float32,
                          kind="ExternalInput")
    out_d = nc.dram_tensor("out", [64, nbatch], mybir.dt.float32,
                           kind="ExternalOutput")
    state_d = nc.dram_tensor("state", [64, nseq], mybir.dt.bfloat16,
                             kind="ExternalOutput")

    FP32 = mybir.dt.float32
    BF = mybir.dt.bfloat16
    SIG = mybir.ActivationFunctionType.Sigmoid
    TANH = mybir.ActivationFunctionType.Tanh

    with tile.TileContext(nc) as tc:
        with tc.tile_pool(name="const", bufs=1) as cpool:
          with tc.tile_pool(name="gbuf", bufs=2) as gpool, \
               tc.tile_pool(name="work", bufs=2) as wpool, \
               tc.tile_pool(name="zps", bufs=2, space="PSUM") as zpool:

            wf = cpool.tile([KDIM, 128], BF)
            nc.sync.dma_start(out=wf[:, :], in_=wf_d.ap())
            # bw weights live at partition base 64: walrus requires matmul
            # fmap and weight to share the same SB start partition, and the
            # bw rhs is G[64:128]. Host pads to [128, 128] (top half zeros)
            # so the DMA itself writes at partition base 0.
            wb_t = cpool.tile([128, 128], BF)
            nc.sync.dma_start(out=wb_t[:, :], in_=wb_d.ap())
            wb = wb_t[64:128, :]
            bv = cpool.tile([128, 4], FP32)
            nc.sync.dma_start(out=bv[:, :], in_=bv_d.ap())
            idx_sb = cpool.tile([128, ntiles * NT * T // 16], mybir.dt.int16)
            nc.sync.dma_start(out=idx_sb[:, :], in_=idx_d.ap())
            state = cpool.tile([64, nseq], BF)

            IDXW = NT * T // 16      # idx columns per tile

            def gather_tile(j):
                g = gpool.tile([128, 1, T * NT], BF, tag=f"g{j % 2}")
                nc.gpsimd.dma_gather(
                    out_ap=g[:, :, :],
                    in_ap=table_d.ap(),
                    idxs_ap=idx_sb[:, j * IDXW:(j + 1) * IDXW],
                    num_idxs=T * NT,
                    num_idxs_reg=T * NT,
                    elem_size=128,
                    transpose=True,
                    single_packet=False,
                )
                return g

            for pair in range(npairs):
                if mode == "empty":
                    break
                if mode == "compute":
                    ga = gpool.tile([128, 1, T * NT], BF, tag="g0",
                                    name=f"ga{pair}")
                    gb = gpool.tile([128, 1, T * NT], BF, tag="g1",
                                    name=f"gb{pair}")
                else:
                    ga = gather_tile(2 * pair)
                    gb = gather_tile(2 * pair + 1)
                gs = [ga, gb]
                if mode == "gather":
                    # consume G so nothing gets dead-code-eliminated
                    for gi2, g_t in enumerate(gs):
                        col0 = (2 * pair + gi2) * NT
                        nc.vector.tensor_copy(
                            state[0:32, col0:col0 + NT],
                            g_t[32:64, 0, (T - 1) * NT:T * NT])
                        nc.vector.tensor_copy(
                            state[32:64, col0:col0 + NT],
                            g_t[64:96, 0, (T - 1) * NT:T * NT])
                    continue
                c_all = wpool.tile([128, NT], FP32, tag="c")

                def sl(off):
                    if isinstance(off, int):
                        return slice(off, off + NT)
                    return bass.ds(off, NT)

                def mm_gate(zt_g, gi, fw_off, bw_off):
                    # 4 streams stacked on PSUM partition quarters
                    for s in range(4):
                        g_t = gs[s // 2]
                        bw = s % 2
                        lo = EMB1 if bw else HFW0
                        w_s = wb if bw else wf
                        rhs = g_t[lo:lo + KDIM, 0, sl(bw_off if bw else fw_off)]
                        nc.tensor.matmul(
                            zt_g[32 * s:32 * s + 32, :],
                            w_s[:, 32 * gi:32 * gi + 32], rhs,
                            start=True, stop=True,
                            tile_position=(64 if bw else 0, 32 * s))

                def lstm_step(fw_off, bw_off, first, last, key):
                    # one PSUM bank per gate, stream s on partition quarter s.
                    # gate issue order g,i,f,o lets each ACT start after only
                    # 4 matmuls and keeps the c-chain off the critical path.
                    zt = [zpool.tile([128, NT], FP32, tag=f"z{gi}",
                                     name=f"z{gi}_{pair}_{key}")
                          for gi in range(4)]
                    mm_gate(zt[2][:, :], 2, fw_off, bw_off)
                    g_all = wpool.tile([128, NT], BF, tag="gall")
                    nc.scalar.activation(g_all[:, :], zt[2][:, :], TANH,
                                         bias=bv[:, 2:3])
                    mm_gate(zt[0][:, :], 0, fw_off, bw_off)
                    ui = wpool.tile([128, NT], BF, tag="ui")
                    nc.scalar.activation(ui[:, :], zt[0][:, :], SIG,
                                         bias=bv[:, 0:1])
                    mm_gate(zt[1][:, :], 1, fw_off, bw_off)
                    uf = wpool.tile([128, NT], BF, tag="uf")
                    nc.scalar.activation(uf[:, :], zt[1][:, :], SIG,
                                         bias=bv[:, 1:2])
                    mm_gate(zt[3][:, :], 3, fw_off, bw_off)
                    uo = wpool.tile([128, NT], BF, tag="uo")
                    nc.scalar.activation(uo[:, :], zt[3][:, :], SIG,
                                         bias=bv[:, 3:4])

                    if first:
                        # c = i * g  (h-slots and previous c are zero)
                        nc.vector.tensor_mul(c_all[:, :], ui[:, :],
                                             g_all[:, :])
                    else:
                        t1 = wpool.tile([128, NT], BF, tag="t1")
                        nc.vector.tensor_mul(t1[:, :], ui[:, :], g_all[:, :])
                        t2 = wpool.tile([128, NT], FP32, tag="t2")
                        nc.vector.tensor_mul(t2[:, :], uf[:, :], c_all[:, :])
                        nc.vector.tensor_add(c_all[:, :], t1[:, :], t2[:, :])

                    tc_t = wpool.tile([128, NT], BF, tag="tc")
                    nc.scalar.activation(tc_t[:, :], c_all[:, :], TANH)

                    # h = o * tanh(c), written straight into the next step's
                    # h-slot of G (or the state tile at the last step)
                    for s in range(4):
                        g_t = gs[s // 2]
                        bw = s % 2
                        if last:
                            col0 = (2 * pair + s // 2) * NT
                            dst = state[32 * bw:32 * bw + 32,
                                        col0:col0 + NT]
                        else:
                            h0 = HBW0 if bw else HFW0
                            hoff = (bw_off - NT) if bw else (fw_off + NT)
                            dst = g_t[h0:h0 + 32, 0, sl(hoff)]
                        nc.vector.tensor_mul(dst,
                                             uo[32 * s:32 * s + 32, :],
                                             tc_t[32 * s:32 * s + 32, :])

                if hw_loop:
                    lstm_step(0, (T - 1) * NT, True, False, "t0")
                    with tc.For_i(NT, (T - 1) * NT, NT,
                                  name=f"tau{pair}") as iv:
                        lstm_step(iv, (T - 1) * NT - iv, False, False, "dyn")
                    lstm_step((T - 1) * NT, 0, False, True, "tZ")
                else:
                    for tau in range(T):
                        lstm_step(tau * NT, (T - 1 - tau) * NT,
                                  tau == 0, tau == T - 1, tau)

          # ---- dense head: out[c, b] = tanh(sum_p Wd_p.T @ state_p + bd)
          if mode == "empty":
            # overhead-floor variant: touch outputs without the LSTM loop
            nc.sync.dma_start(out=out_d.ap()[0:64, 0:4], in_=bv[0:64, 0:4])
            nc.sync.dma_start(out=state_d.ap()[0:64, 0:128],
                              in_=wb_t[0:64, 0:128])
          else:
            wd = cpool.tile([64, 4096], BF)
            nc.sync.dma_start(out=wd[:, :], in_=wd_d.ap())
            bd = cpool.tile([64, 1], FP32)
            nc.sync.dma_start(out=bd[:, :], in_=bd_d.ap())

            with tc.tile_pool(name="head", bufs=1, space="PSUM") as hpool:
                st_r = state[:, :].rearrange("p (b q) -> p q b", q=P)
                hp = hpool.tile([64, nbatch], FP32)
                for p in range(P):
                    nc.tensor.matmul(hp[:, :], wd[:, 64 * p:64 * p + 64],
                                     st_r[:, p:p + 1, :],
                                     start=(p == 0), stop=(p == P - 1))
                out_sb = cpool.tile([64, nbatch], FP32)
                nc.scalar.activation(out_sb[:, :], hp[:, :], TANH,
                                     bias=bd[:, :])
                nc.sync.dma_start(out=out_d.ap(), in_=out_sb[:, :])
                nc.sync.dma_start(out=state_d.ap(), in_=state[:, :])

    nc.compile()
    return nc


# ---------------------------------------------------------------------------
# host-side packing
# ---------------------------------------------------------------------------

def pack_table(embed_table):
    tbl = np.zeros((VOCP, 128), np.float32)
    tbl[:VOC, EMB0:EMB0 + 32] = _f32(embed_table)
    tbl[:VOC, EMB1:EMB1 + 32] = _f32(embed_table)
    return tbl.astype(BF16)


def pack_idx(x_core, nseq=NSEQ):
    """x_core: [nseq, T] int32 -> wrapped int16 [128, ntiles * T*NT/16]."""
    ntiles = nseq // NT
    cols = []
    for j in range(ntiles):
        u = x_core[j * NT:(j + 1) * NT, :].T.reshape(-1)     # t-major [T*NT]
        w = u.reshape(-1, 16).T                               # [16, T*NT/16]
        cols.append(np.tile(w, (8, 1)))
    return np.concatenate(cols, axis=1).astype(np.int16)


def pack_weights(Wk, Wr, b):
    Wk, Wr, b = _f32(Wk), _f32(Wr), _f32(b)
    wf = np.concatenate([Wr, Wk], 0)                          # [64, 128]
    wb = np.concatenate([np.zeros((64, 128), np.float32), Wk, Wr], 0)
    bv = np.tile(b.reshape(4, 32), (1, 4)).reshape(4, 128).T  # [128, 4]
    return wf.astype(BF16), wb.astype(BF16), np.ascontiguousarray(bv, np.float32)


def pack_wd(Wd):
    w = _f32(Wd).reshape(P, 64, 64).transpose(1, 0, 2).reshape(64, 4096)
    return w.astype(BF16)


# ---------------------------------------------------------------------------
# host reference bits for the zero-token fixup
# ---------------------------------------------------------------------------

def _np_lstm_last_h(emb, mask, Wk, Wr, b):
    n = emb.shape[0]
    h = np.zeros((n, H), np.float32)
    c = np.zeros((n, H), np.float32)
    for t in range(emb.shape[1]):
        z = emb[:, t, :] @ Wk + h @ Wr + b
        i = 1.0 / (1.0 + np.exp(-z[:, 0:32]))
        f = 1.0 / (1.0 + np.exp(-z[:, 32:64]))
        g = np.tanh(z[:, 64:96])
        o = 1.0 / (1.0 + np.exp(-z[:, 96:128]))
        c_new = f * c + i * g
        h_new = o * np.tanh(c_new)
        m = mask[:, t][:, None]
        h = np.where(m, h_new, h)
        c = np.where(m, c_new, c)
    return h


def _host_fixup(out, state_all, x_flat, embed_table, Wk, Wr, b, Wd, bd):
    """Recompute rows whose sequences contain a zero token."""
    mask = x_flat != 0
    bad_seq = np.nonzero(~mask.all(axis=1))[0]
    if bad_seq.size == 0:
        return out
    emb = _f32(embed_table)[x_flat[bad_seq]]
    h_fw = _np_lstm_last_h(emb, mask[bad_seq], _f32(Wk), _f32(Wr), _f32(b))
    h_bw = _np_lstm_last_h(emb[:, ::-1, :], mask[bad_seq][:, ::-1],
                           _f32(Wk), _f32(Wr), _f32(b))
    state_all = state_all.copy()
    state_all[bad_seq] = np.concatenate([h_fw, h_bw], axis=1)
    bad_rows = np.unique(bad_seq // P)
    st = state_all[bad_rows[:, None] * P + np.arange(P)[None, :]]
    st = st.reshape(bad_rows.size, P * 64)
    out[bad_rows] = np.tanh(st @ _f32(Wd) + _f32(bd))
    return out


# ---------------------------------------------------------------------------
# entry point
# ---------------------------------------------------------------------------

_NC_CACHE = {}


def _get_nc(mode="full"):
    key = "nc" + mode
    if key not in _NC_CACHE:
        _NC_CACHE[key] = build_kernel(mode=mode)
    return _NC_CACHE[key]


def run_device(inputs, trace=False):
    x = np.asarray(inputs["x"])
    table = pack_table(inputs["embed_table"])
    wf, wb, bv = pack_weights(inputs["Wk"], inputs["Wr"], inputs["b"])
    wd = pack_wd(inputs["Wd"])
    bd = _f32(inputs["bd"]).reshape(64, 1)

    x_flat = x.reshape(B * P, L)
    in_maps = []
    for k in range(N_CORES):
        x_core = x_flat[k * NSEQ:(k + 1) * NSEQ]
        in_maps.append({
            "table": table,
            "idx": pack_idx(x_core),
            "wf": wf,
            "wb": wb,
            "bv": bv,
            "wd": wd,
            "bd": bd,
        })

    nc = _get_nc()
    res = bass_utils.run_bass_kernel_spmd(
        nc, in_maps, core_ids=list(range(N_CORES)), trace=trace)

    out = np.empty((B, 64), np.float32)
    state_all = np.empty((B * P, 64), np.float32)
    for k in range(N_CORES):
        out[k * B_LOC:(k + 1) * B_LOC] = res.results[k]["out"].T
        state_all[k * NSEQ:(k + 1) * NSEQ] = \
            _f32(res.results[k]["state"]).T
    return out, state_all, res


def kernel(x, embed_table, Wk, Wr, b, Wd, bd):
    inputs = dict(x=x, embed_table=embed_table, Wk=Wk, Wr=Wr, b=b,
                  Wd=Wd, bd=bd)
    out, state_all, _ = run_device(inputs)
    out = _host_fixup(out, state_all, np.asarray(x).reshape(B * P, L),
                      embed_table, Wk, Wr, b, Wd, bd)
    return out



# revision 13
# speedup vs baseline: 1.6783x; 1.5248x over previous
"""CardEncoder Trainium2 kernel.

Model (per sequence of L=16 tokens): embed(32) -> bidirectional LSTM(32) ->
concat final states -> per-batch dense (4096 -> 64) -> tanh.

Strategy (pure data parallel, 8 cores, batch-sharded):
  * Host packs an augmented gather table [10112, 128] bf16 per vocab row:
      [ h_fw slot (zeros 0:32) | embedding (32:64) | 1.0 bias (64) |
        h_bw slot (zeros 65:97) | zeros ]
  * Device gathers rows with dma_gather(transpose=True) -> feature-major
    tiles G[128, T*NT] (columns = (t, seq)); the constant-1 row turns the
    LSTM bias into a matmul row; the zero h-slots are overwritten in-place
    with the running hidden state so each step is ONE K=65 matmul per gate.
  * 4 streams (2 seq-tiles x {fw, bw}) stacked on PSUM partition quarters via
    tile_position col-groups, so all elementwise work runs on full
    [128, NT] tiles.
  * LSTM cell: i,f,o = sigmoid, g = tanh (ACT); c = f*c + i*g, h = o*tanh(c)
    (DVE); h copied into the next step's h-slot of G.
  * Dense head on device; host transposes [64, B] -> [B, 64].
  * mask_zero=True handling: token==0 steps must leave (h, c) unchanged.
    Zero tokens occur w.p. 1e-4; the device ignores masking and the host
    recomputes the ~0.02% of affected sequences exactly and patches the
    affected output rows.
"""

import os
import numpy as np
import ml_dtypes

os.environ.setdefault("JAX_PLATFORMS", "cpu")

import concourse.bass as bass
import concourse.bacc as bacc
import concourse.mybir as mybir
import concourse.tile as tile
from concourse import bass_utils

BF16 = ml_dtypes.bfloat16

B, P, L = 2048, 64, 16
H = 32                      # LSTM units per direction
VOC = 10000
VOCP = 10112                # padded to 79 * 128
N_CORES = 8
B_LOC = B // N_CORES        # 256 batches per core
NSEQ = B_LOC * P            # 16384 sequences per core
NT = 512                    # sequences per tile
T = L

# G tile partition layout (SBUF APs must start at partition 0/32/64/96):
#   [ h_fw slot 0:32 | emb 32:64 | emb copy 64:96 | h_bw slot 96:128 ]
# fw rhs = rows 0:64 [h_fw, emb];  bw rhs = rows 64:128 [emb, h_bw].
HFW0 = 0
EMB0 = 32
EMB1 = 64
HBW0 = 96
KDIM = 64                   # matmul contraction size


def _f32(x):
    return np.asarray(x, np.float32)


# ---------------------------------------------------------------------------
# device kernel
# ---------------------------------------------------------------------------

def build_kernel(nseq=NSEQ, mode="full", hw_loop=False):
    ntiles = nseq // NT
    npairs = ntiles // 2
    nbatch = nseq // P

    nc = bacc.Bacc("TRN2", target_bir_lowering=False, debug=False,
                   enable_asserts=False, num_devices=N_CORES)

    table_d = nc.dram_tensor("table", [VOCP, 128], mybir.dt.bfloat16,
                             kind="ExternalInput")
    idx_d = nc.dram_tensor("idx", [128, ntiles * NT * T // 16], mybir.dt.int16,
                           kind="ExternalInput")
    wf_d = nc.dram_tensor("wf", [KDIM, 128], mybir.dt.bfloat16,
                          kind="ExternalInput")
    wb_d = nc.dram_tensor("wb", [128, 128], mybir.dt.bfloat16,
                          kind="ExternalInput")
    bv_d = nc.dram_tensor("bv", [128, 4], mybir.dt.float32,
                          kind="ExternalInput")
    wd_d = nc.dram_tensor("wd", [64, 4096], mybir.dt.bfloat16,
                          kind="ExternalInput")
    bd_d = nc.dram_tensor("bd", [64, 1], mybir.dt.float32,
                          kind="ExternalInput")
    out_d = nc.dram_tensor("out", [64, nbatch], mybir.dt.float32,
                           kind="ExternalOutput")

    FP32 = mybir.dt.float32
    BF = mybir.dt.bfloat16
    SIG = mybir.ActivationFunctionType.Sigmoid
    TANH = mybir.ActivationFunctionType.Tanh

    with tile.TileContext(nc) as tc:
        with tc.tile_pool(name="const", bufs=1) as cpool:
          with tc.tile_pool(name="gbuf", bufs=2) as gpool, \
               tc.tile_pool(name="work", bufs=2) as wpool, \
               tc.tile_pool(name="zps", bufs=2, space="PSUM") as zpool:

            wf = cpool.tile([KDIM, 128], BF)
            nc.sync.dma_start(out=wf[:, :], in_=wf_d.ap())
            # bw weights live at partition base 64: walrus requires matmul
            # fmap and weight to share the same SB start partition, and the
            # bw rhs is G[64:128]. Host pads to [128, 128] (top half zeros)
            # so the DMA itself writes at partition base 0.
            wb_t = cpool.tile([128, 128], BF)
            nc.sync.dma_start(out=wb_t[:, :], in_=wb_d.ap())
            wb = wb_t[64:128, :]
            bv = cpool.tile([128, 4], FP32)
            nc.sync.dma_start(out=bv[:, :], in_=bv_d.ap())
            idx_sb = cpool.tile([128, ntiles * NT * T // 16], mybir.dt.int16)
            nc.sync.dma_start(out=idx_sb[:, :], in_=idx_d.ap())
            state = cpool.tile([64, nseq], BF)

            IDXW = NT * T // 16      # idx columns per tile

            def gather_tile(j):
                g = gpool.tile([128, 1, T * NT], BF, tag=f"g{j % 2}")
                nc.gpsimd.dma_gather(
                    out_ap=g[:, :, :],
                    in_ap=table_d.ap(),
                    idxs_ap=idx_sb[:, j * IDXW:(j + 1) * IDXW],
                    num_idxs=T * NT,
                    num_idxs_reg=T * NT,
                    elem_size=128,
                    transpose=True,
                    single_packet=False,
                )
                return g

            for pair in range(npairs):
                if mode == "empty":
                    break
                if mode == "compute":
                    ga = gpool.tile([128, 1, T * NT], BF, tag="g0",
                                    name=f"ga{pair}")
                    gb = gpool.tile([128, 1, T * NT], BF, tag="g1",
                                    name=f"gb{pair}")
                else:
                    ga = gather_tile(2 * pair)
                    gb = gather_tile(2 * pair + 1)
                gs = [ga, gb]
                if mode == "gather":
                    # consume G so nothing gets dead-code-eliminated
                    for gi2, g_t in enumerate(gs):
                        col0 = (2 * pair + gi2) * NT
                        nc.vector.tensor_copy(
                            state[0:32, col0:col0 + NT],
                            g_t[32:64, 0, (T - 1) * NT:T * NT])
                        nc.vector.tensor_copy(
                            state[32:64, col0:col0 + NT],
                            g_t[64:96, 0, (T - 1) * NT:T * NT])
                    continue
                c_all = wpool.tile([128, NT], FP32, tag="c")

                def sl(off):
                    if isinstance(off, int):
                        return slice(off, off + NT)
                    return bass.ds(off, NT)

                def mm_gate(zt_g, gi, fw_off, bw_off):
                    # 4 streams stacked on PSUM partition quarters
                    for s in range(4):
                        g_t = gs[s // 2]
                        bw = s % 2
                        lo = EMB1 if bw else HFW0
                        w_s = wb if bw else wf
                        rhs = g_t[lo:lo + KDIM, 0, sl(bw_off if bw else fw_off)]
                        nc.tensor.matmul(
                            zt_g[32 * s:32 * s + 32, :],
                            w_s[:, 32 * gi:32 * gi + 32], rhs,
                            start=True, stop=True,
                            tile_position=(64 if bw else 0, 32 * s))

                def lstm_step(fw_off, bw_off, first, last, key):
                    # one PSUM bank per gate, stream s on partition quarter s.
                    # gate issue order g,i,f,o lets each ACT start after only
                    # 4 matmuls and keeps the c-chain off the critical path.
                    zt = [zpool.tile([128, NT], FP32, tag=f"z{gi}",
                                     name=f"z{gi}_{pair}_{key}")
                          for gi in range(4)]
                    mm_gate(zt[2][:, :], 2, fw_off, bw_off)
                    g_all = wpool.tile([128, NT], BF, tag="gall")
                    nc.scalar.activation(g_all[:, :], zt[2][:, :], TANH,
                                         bias=bv[:, 2:3])
                    mm_gate(zt[0][:, :], 0, fw_off, bw_off)
                    ui = wpool.tile([128, NT], BF, tag="ui")
                    nc.scalar.activation(ui[:, :], zt[0][:, :], SIG,
                                         bias=bv[:, 0:1])
                    mm_gate(zt[1][:, :], 1, fw_off, bw_off)
                    uf = wpool.tile([128, NT], BF, tag="uf")
                    nc.scalar.activation(uf[:, :], zt[1][:, :], SIG,
                                         bias=bv[:, 1:2])
                    mm_gate(zt[3][:, :], 3, fw_off, bw_off)
                    uo = wpool.tile([128, NT], BF, tag="uo")
                    nc.scalar.activation(uo[:, :], zt[3][:, :], SIG,
                                         bias=bv[:, 3:4])

                    if first:
                        # c = i * g  (h-slots and previous c are zero)
                        nc.vector.tensor_mul(c_all[:, :], ui[:, :],
                                             g_all[:, :])
                    else:
                        t1 = wpool.tile([128, NT], BF, tag="t1")
                        nc.vector.tensor_mul(t1[:, :], ui[:, :], g_all[:, :])
                        t2 = wpool.tile([128, NT], FP32, tag="t2")
                        nc.vector.tensor_mul(t2[:, :], uf[:, :], c_all[:, :])
                        nc.vector.tensor_add(c_all[:, :], t1[:, :], t2[:, :])

                    tc_t = wpool.tile([128, NT], BF, tag="tc")
                    nc.scalar.activation(tc_t[:, :], c_all[:, :], TANH)

                    # h = o * tanh(c), written straight into the next step's
                    # h-slot of G (or the state tile at the last step)
                    for s in range(4):
                        g_t = gs[s // 2]
                        bw = s % 2
                        if last:
                            col0 = (2 * pair + s // 2) * NT
                            dst = state[32 * bw:32 * bw + 32,
                                        col0:col0 + NT]
                        else:
                            h0 = HBW0 if bw else HFW0
                            hoff = (bw_off - NT) if bw else (fw_off + NT)
                            dst = g_t[h0:h0 + 32, 0, sl(hoff)]
                        nc.vector.tensor_mul(dst,
                                             uo[32 * s:32 * s + 32, :],
                                             tc_t[32 * s:32 * s + 32, :])

                if hw_loop:
                    lstm_step(0, (T - 1) * NT, True, False, "t0")
                    with tc.For_i(NT, (T - 1) * NT, NT,
                                  name=f"tau{pair}") as iv:
                        lstm_step(iv, (T - 1) * NT - iv, False, False, "dyn")
                    lstm_step((T - 1) * NT, 0, False, True, "tZ")
                else:
                    for tau in range(T):
                        lstm_step(tau * NT, (T - 1 - tau) * NT,
                                  tau == 0, tau == T - 1, tau)

          # ---- dense head: out[c, b] = tanh(sum_p Wd_p.T @ state_p + bd)
          if mode == "empty":
            # overhead-floor variant: touch outputs without the LSTM loop
            nc.sync.dma_start(out=out_d.ap()[0:64, 0:4], in_=bv[0:64, 0:4])
          else:
            wd = cpool.tile([64, 4096], BF)
            nc.sync.dma_start(out=wd[:, :], in_=wd_d.ap())
            bd = cpool.tile([64, 1], FP32)
            nc.sync.dma_start(out=bd[:, :], in_=bd_d.ap())

            with tc.tile_pool(name="head", bufs=1, space="PSUM") as hpool:
                st_r = state[:, :].rearrange("p (b q) -> p q b", q=P)
                hp = hpool.tile([64, nbatch], FP32)
                for p in range(P):
                    nc.tensor.matmul(hp[:, :], wd[:, 64 * p:64 * p + 64],
                                     st_r[:, p:p + 1, :],
                                     start=(p == 0), stop=(p == P - 1))
                out_sb = cpool.tile([64, nbatch], FP32)
                nc.scalar.activation(out_sb[:, :], hp[:, :], TANH,
                                     bias=bd[:, :])
                nc.sync.dma_start(out=out_d.ap(), in_=out_sb[:, :])

    nc.compile()
    return nc


def build_kernel_pairloop(nseq=NSEQ):
    """Hardware-loop variant: one For_i over pairs of seq-tiles, LSTM time
    loop unrolled inside the body.  All register-offset APs are partition-
    base-0 (base!=0 + register offset is broken in lowering): the final
    hidden states go to separate [32, nseq] fw/bw tiles, and per-pair index
    slices are DMA-staged from DRAM at dynamic offsets."""
    from concourse.expressions import smin

    ntiles = nseq // NT
    npairs = ntiles // 2
    nbatch = nseq // P
    IDXW = NT * T // 16
    PCOLS = 2 * IDXW              # idx cols per pair

    nc = bacc.Bacc("TRN2", target_bir_lowering=False, debug=False,
                   enable_asserts=False, num_devices=N_CORES)

    table_d = nc.dram_tensor("table", [VOCP, 128], mybir.dt.bfloat16,
                             kind="ExternalInput")
    idx_d = nc.dram_tensor("idx", [128, ntiles * IDXW], mybir.dt.int16,
                           kind="ExternalInput")
    wf_d = nc.dram_tensor("wf", [KDIM, 128], mybir.dt.bfloat16,
                          kind="ExternalInput")
    wb_d = nc.dram_tensor("wb", [128, 128], mybir.dt.bfloat16,
                          kind="ExternalInput")
    bv_d = nc.dram_tensor("bv", [128, 4], mybir.dt.float32,
                          kind="ExternalInput")
    wd_d = nc.dram_tensor("wd", [64, 4096], mybir.dt.bfloat16,
                          kind="ExternalInput")
    bd_d = nc.dram_tensor("bd", [64, 1], mybir.dt.float32,
                          kind="ExternalInput")
    out_d = nc.dram_tensor("out", [64, nbatch], mybir.dt.float32,
                           kind="ExternalOutput")

    FP32 = mybir.dt.float32
    BF = mybir.dt.bfloat16
    SIG = mybir.ActivationFunctionType.Sigmoid
    TANH = mybir.ActivationFunctionType.Tanh

    with tile.TileContext(nc) as tc:
      with tc.tile_pool(name="const", bufs=1) as cpool:
        wf = cpool.tile([KDIM, 128], BF)
        nc.sync.dma_start(out=wf[:, :], in_=wf_d.ap())
        wb_t = cpool.tile([128, 128], BF)
        nc.sync.dma_start(out=wb_t[:, :], in_=wb_d.ap())
        wb = wb_t[64:128, :]
        bv = cpool.tile([128, 4], FP32)
        nc.sync.dma_start(out=bv[:, :], in_=bv_d.ap())
        st_fw = cpool.tile([32, nseq], BF)
        st_bw = cpool.tile([32, nseq], BF)

        with tc.tile_pool(name="gbuf", bufs=1) as gpool, \
             tc.tile_pool(name="ibuf", bufs=2) as ipool, \
             tc.tile_pool(name="work", bufs=2) as wpool, \
             tc.tile_pool(name="zps", bufs=2, space="PSUM") as zpool:

            # two static double-buffer sets; For_i bodies can't rotate pools
            g_set = {k: [gpool.tile([128, 1, T * NT], BF, name=f"g{k}{j}")
                         for j in range(2)] for k in "AB"}

            def load_and_gather(k, idx_off, key):
                idx_sb = ipool.tile([128, PCOLS], mybir.dt.int16,
                                    tag=f"i{k}", name=f"idx{k}_{key}")
                nc.sync.dma_start(out=idx_sb[:, :],
                                  in_=idx_d.ap()[:, bass.ds(idx_off, PCOLS)])
                for j in range(2):
                    nc.gpsimd.dma_gather(
                        out_ap=g_set[k][j][:, :, :],
                        in_ap=table_d.ap(),
                        idxs_ap=idx_sb[:, j * IDXW:(j + 1) * IDXW],
                        num_idxs=T * NT,
                        num_idxs_reg=T * NT,
                        elem_size=128,
                        transpose=True,
                        single_packet=False,
                    )

            def lstm_pair(k, st_off, key):
                gs = g_set[k]
                c_all = wpool.tile([128, NT], FP32, tag=f"c{k}")

                def mm_gate(zt_g, gi, tau):
                    for s in range(4):
                        g_t = gs[s // 2]
                        bw = s % 2
                        blk = (T - 1 - tau) if bw else tau
                        lo = EMB1 if bw else HFW0
                        w_s = wb if bw else wf
                        rhs = g_t[lo:lo + KDIM, 0, blk * NT:(blk + 1) * NT]
                        nc.tensor.matmul(
                            zt_g[32 * s:32 * s + 32, :],
                            w_s[:, 32 * gi:32 * gi + 32], rhs,
                            start=True, stop=True,
                            tile_position=(64 if bw else 0, 32 * s))

                for tau in range(T):
                    zt = [zpool.tile([128, NT], FP32, tag=f"z{gi}",
                                     name=f"z{gi}_{key}_{tau}")
                          for gi in range(4)]
                    mm_gate(zt[2][:, :], 2, tau)
                    g_all = wpool.tile([128, NT], BF, tag="gall")
                    nc.scalar.activation(g_all[:, :], zt[2][:, :], TANH,
                                         bias=bv[:, 2:3])
                    mm_gate(zt[0][:, :], 0, tau)
                    ui = wpool.tile([128, NT], BF, tag="ui")
                    nc.scalar.activation(ui[:, :], zt[0][:, :], SIG,
                                         bias=bv[:, 0:1])
                    mm_gate(zt[1][:, :], 1, tau)
                    uf = wpool.tile([128, NT], BF, tag="uf")
                    nc.scalar.activation(uf[:, :], zt[1][:, :], SIG,
                                         bias=bv[:, 1:2])
                    mm_gate(zt[3][:, :], 3, tau)
                    uo = wpool.tile([128, NT], BF, tag="uo")
                    nc.scalar.activation(uo[:, :], zt[3][:, :], SIG,
                                         bias=bv[:, 3:4])

                    if tau == 0:
                        nc.vector.tensor_mul(c_all[:, :], ui[:, :],
                                             g_all[:, :])
                    else:
                        t1 = wpool.tile([128, NT], BF, tag="t1")
                        nc.vector.tensor_mul(t1[:, :], ui[:, :], g_all[:, :])
                        t2 = wpool.tile([128, NT], FP32, tag="t2")
                        nc.vector.tensor_mul(t2[:, :], uf[:, :], c_all[:, :])
                        nc.vector.tensor_add(c_all[:, :], t1[:, :],
                                             t2[:, :])

                    tc_t = wpool.tile([128, NT], BF, tag="tc")
                    nc.scalar.activation(tc_t[:, :], c_all[:, :], TANH)

                    for s in range(4):
                        g_t = gs[s // 2]
                        bw = s % 2
                        if tau == T - 1:
                            st = st_bw if bw else st_fw
                            dst = st[0:32,
                                     bass.ds(st_off + (s // 2) * NT, NT)]
                        else:
                            nblk = (T - 2 - tau) if bw else (tau + 1)
                            h0 = HBW0 if bw else HFW0
                            dst = g_t[h0:h0 + 32, 0,
                                      nblk * NT:(nblk + 1) * NT]
                        nc.vector.tensor_mul(dst,
                                             uo[32 * s:32 * s + 32, :],
                                             tc_t[32 * s:32 * s + 32, :])

            load_and_gather("A", 0, "p0")
            load_and_gather("B", PCOLS, "p1")
            LAST_A = (npairs - 2) * PCOLS
            LAST_B = (npairs - 1) * PCOLS
            with tc.For_i(0, npairs * PCOLS, 2 * PCOLS,
                          name="pairs") as iv:
                lstm_pair("A", iv, "A")
                load_and_gather("A", smin(iv + 2 * PCOLS, LAST_A), "nA")
                lstm_pair("B", iv + PCOLS, "B")
                load_and_gather("B", smin(iv + 3 * PCOLS, LAST_B), "nB")

        # ---- dense head: split fw/bw contraction, K=32 each
        wd_fw = cpool.tile([32, 4096], BF)
        nc.sync.dma_start(out=wd_fw[:, :], in_=wd_d.ap()[0:32, :])
        wd_bw = cpool.tile([32, 4096], BF)
        nc.sync.dma_start(out=wd_bw[:, :], in_=wd_d.ap()[32:64, :])
        bd = cpool.tile([64, 1], FP32)
        nc.sync.dma_start(out=bd[:, :], in_=bd_d.ap())

        with tc.tile_pool(name="head", bufs=1, space="PSUM") as hpool:
            fw_r = st_fw[:, :].rearrange("p (b q) -> p q b", q=P)
            bw_r = st_bw[:, :].rearrange("p (b q) -> p q b", q=P)
            hp = hpool.tile([64, nbatch], FP32)
            for p in range(P):
                nc.tensor.matmul(hp[:, :], wd_fw[:, 64 * p:64 * p + 64],
                                 fw_r[:, p:p + 1, :],
                                 start=(p == 0), stop=False,
                                 tile_position=(0, 0))
            for p in range(P):
                nc.tensor.matmul(hp[:, :], wd_bw[:, 64 * p:64 * p + 64],
                                 bw_r[:, p:p + 1, :],
                                 start=False, stop=(p == P - 1),
                                 tile_position=(0, 0))
            out_sb = cpool.tile([64, nbatch], FP32)
            nc.scalar.activation(out_sb[:, :], hp[:, :], TANH,
                                 bias=bd[:, :])
            nc.sync.dma_start(out=out_d.ap(), in_=out_sb[:, :])

    nc.compile()
    return nc


# ---------------------------------------------------------------------------
# host-side packing
# ---------------------------------------------------------------------------

def pack_table(embed_table):
    tbl = np.zeros((VOCP, 128), np.float32)
    tbl[:VOC, EMB0:EMB0 + 32] = _f32(embed_table)
    tbl[:VOC, EMB1:EMB1 + 32] = _f32(embed_table)
    return tbl.astype(BF16)


def pack_idx(x_core, nseq=NSEQ):
    """x_core: [nseq, T] int32 -> wrapped int16 [128, ntiles * T*NT/16]."""
    ntiles = nseq // NT
    cols = []
    for j in range(ntiles):
        u = x_core[j * NT:(j + 1) * NT, :].T.reshape(-1)     # t-major [T*NT]
        w = u.reshape(-1, 16).T                               # [16, T*NT/16]
        cols.append(np.tile(w, (8, 1)))
    return np.concatenate(cols, axis=1).astype(np.int16)


def pack_weights(Wk, Wr, b):
    Wk, Wr, b = _f32(Wk), _f32(Wr), _f32(b)
    wf = np.concatenate([Wr, Wk], 0)                          # [64, 128]
    wb = np.concatenate([np.zeros((64, 128), np.float32), Wk, Wr], 0)
    bv = np.tile(b.reshape(4, 32), (1, 4)).reshape(4, 128).T  # [128, 4]
    return wf.astype(BF16), wb.astype(BF16), np.ascontiguousarray(bv, np.float32)


def pack_wd(Wd):
    w = _f32(Wd).reshape(P, 64, 64).transpose(1, 0, 2).reshape(64, 4096)
    return w.astype(BF16)


# ---------------------------------------------------------------------------
# host reference bits for the zero-token fixup
# ---------------------------------------------------------------------------

def _np_lstm_last_h(emb, mask, Wk, Wr, b):
    n = emb.shape[0]
    h = np.zeros((n, H), np.float32)
    c = np.zeros((n, H), np.float32)
    for t in range(emb.shape[1]):
        z = emb[:, t, :] @ Wk + h @ Wr + b
        i = 1.0 / (1.0 + np.exp(-z[:, 0:32]))
        f = 1.0 / (1.0 + np.exp(-z[:, 32:64]))
        g = np.tanh(z[:, 64:96])
        o = 1.0 / (1.0 + np.exp(-z[:, 96:128]))
        c_new = f * c + i * g
        h_new = o * np.tanh(c_new)
        m = mask[:, t][:, None]
        h = np.where(m, h_new, h)
        c = np.where(m, c_new, c)
    return h


def _host_fixup(out, x_flat, embed_table, Wk, Wr, b, Wd, bd):
    """Recompute batch rows whose sequences contain a zero token.

    The device ignores mask_zero (tokens are zero w.p. 1e-4); affected rows
    (~0.1% of the batch) are recomputed exactly on the host, including every
    path of each affected row, so no device state output is needed."""
    mask = x_flat != 0
    bad_seq = np.nonzero(~mask.all(axis=1))[0]
    if bad_seq.size == 0:
        return out
    bad_rows = np.unique(bad_seq // P)
    seqs = (bad_rows[:, None] * P + np.arange(P)[None, :]).reshape(-1)
    emb = _f32(embed_table)[x_flat[seqs]]
    m = mask[seqs]
    h_fw = _np_lstm_last_h(emb, m, _f32(Wk), _f32(Wr), _f32(b))
    h_bw = _np_lstm_last_h(emb[:, ::-1, :], m[:, ::-1],
                           _f32(Wk), _f32(Wr), _f32(b))
    st = np.concatenate([h_fw, h_bw], axis=1)
    st = st.reshape(bad_rows.size, P * 64)
    out[bad_rows] = np.tanh(st @ _f32(Wd) + _f32(bd))
    return out


# ---------------------------------------------------------------------------
# entry point
# ---------------------------------------------------------------------------

_NC_CACHE = {}


def _get_nc(mode="full"):
    hw_loop = os.environ.get("K_HW_LOOP", "0") == "1"
    key = "nc" + mode + ("hw" if hw_loop else "")
    if key not in _NC_CACHE:
        _NC_CACHE[key] = build_kernel(mode=mode, hw_loop=hw_loop)
    return _NC_CACHE[key]


def run_device(inputs, trace=False):
    x = np.asarray(inputs["x"])
    table = pack_table(inputs["embed_table"])
    wf, wb, bv = pack_weights(inputs["Wk"], inputs["Wr"], inputs["b"])
    wd = pack_wd(inputs["Wd"])
    bd = _f32(inputs["bd"]).reshape(64, 1)

    x_flat = x.reshape(B * P, L)
    in_maps = []
    for k in range(N_CORES):
        x_core = x_flat[k * NSEQ:(k + 1) * NSEQ]
        in_maps.append({
            "table": table,
            "idx": pack_idx(x_core),
            "wf": wf,
            "wb": wb,
            "bv": bv,
            "wd": wd,
            "bd": bd,
        })

    nc = _get_nc()
    res = bass_utils.run_bass_kernel_spmd(
        nc, in_maps, core_ids=list(range(N_CORES)), trace=trace)

    out = np.empty((B, 64), np.float32)
    for k in range(N_CORES):
        out[k * B_LOC:(k + 1) * B_LOC] = res.results[k]["out"].T
    return out, res


def kernel(x, embed_table, Wk, Wr, b, Wd, bd):
    inputs = dict(x=x, embed_table=embed_table, Wk=Wk, Wr=Wr, b=b,
                  Wd=Wd, bd=bd)
    out, _ = run_device(inputs)
    out = _host_fixup(out, np.asarray(x).reshape(B * P, L),
                      embed_table, Wk, Wr, b, Wd, bd)
    return out



# revision 14
# speedup vs baseline: 2.2400x; 1.3347x over previous
"""CardEncoder Trainium2 kernel.

Model (per sequence of L=16 tokens): embed(32) -> bidirectional LSTM(32) ->
concat final states -> per-batch dense (4096 -> 64) -> tanh.

Strategy (pure data parallel, 8 cores, batch-sharded):
  * Host packs an augmented gather table [10112, 128] bf16 per vocab row:
      [ h_fw slot (zeros 0:32) | embedding (32:64) | 1.0 bias (64) |
        h_bw slot (zeros 65:97) | zeros ]
  * Device gathers rows with dma_gather(transpose=True) -> feature-major
    tiles G[128, T*NT] (columns = (t, seq)); the constant-1 row turns the
    LSTM bias into a matmul row; the zero h-slots are overwritten in-place
    with the running hidden state so each step is ONE K=65 matmul per gate.
  * 4 streams (2 seq-tiles x {fw, bw}) stacked on PSUM partition quarters via
    tile_position col-groups, so all elementwise work runs on full
    [128, NT] tiles.
  * LSTM cell: i,f,o = sigmoid, g = tanh (ACT); c = f*c + i*g, h = o*tanh(c)
    (DVE); h copied into the next step's h-slot of G.
  * Dense head on device; host transposes [64, B] -> [B, 64].
  * mask_zero=True handling: token==0 steps must leave (h, c) unchanged.
    Zero tokens occur w.p. 1e-4; the device ignores masking and the host
    recomputes the ~0.02% of affected sequences exactly and patches the
    affected output rows.
"""

import os
import numpy as np
import ml_dtypes

os.environ.setdefault("JAX_PLATFORMS", "cpu")

import concourse.bass as bass
import concourse.bacc as bacc
import concourse.mybir as mybir
import concourse.tile as tile
from concourse import bass_utils

BF16 = ml_dtypes.bfloat16

B, P, L = 2048, 64, 16
H = 32                      # LSTM units per direction
VOC = 10000
VOCP = 10112                # padded to 79 * 128
N_CORES = 8
B_LOC = B // N_CORES        # 256 batches per core
NSEQ = B_LOC * P            # 16384 sequences per core
NT = 512                    # sequences per tile
T = L

# G tile partition layout (SBUF APs must start at partition 0/32/64/96):
#   [ h_fw slot 0:32 | emb 32:64 | emb copy 64:96 | h_bw slot 96:128 ]
# fw rhs = rows 0:64 [h_fw, emb];  bw rhs = rows 64:128 [emb, h_bw].
HFW0 = 0
EMB0 = 32
EMB1 = 64
HBW0 = 96
KDIM = 64                   # matmul contraction size


def _f32(x):
    return np.asarray(x, np.float32)


# ---------------------------------------------------------------------------
# device kernel
# ---------------------------------------------------------------------------

def build_kernel(nseq=NSEQ, mode="full", hw_loop=False):
    ntiles = nseq // NT
    npairs = ntiles // 2
    nbatch = nseq // P

    nc = bacc.Bacc("TRN2", target_bir_lowering=False, debug=False,
                   enable_asserts=False, num_devices=N_CORES)

    table_d = nc.dram_tensor("table", [VOCP, 128], mybir.dt.bfloat16,
                             kind="ExternalInput")
    idx_d = nc.dram_tensor("idx", [128, ntiles * NT * T // 16], mybir.dt.int16,
                           kind="ExternalInput")
    wf_d = nc.dram_tensor("wf", [KDIM, 128], mybir.dt.bfloat16,
                          kind="ExternalInput")
    wb_d = nc.dram_tensor("wb", [128, 128], mybir.dt.bfloat16,
                          kind="ExternalInput")
    bv_d = nc.dram_tensor("bv", [128, 4], mybir.dt.float32,
                          kind="ExternalInput")
    wd_d = nc.dram_tensor("wd", [64, 4096], mybir.dt.bfloat16,
                          kind="ExternalInput")
    bd_d = nc.dram_tensor("bd", [64, 1], mybir.dt.float32,
                          kind="ExternalInput")
    out_d = nc.dram_tensor("out", [64, nbatch], mybir.dt.float32,
                           kind="ExternalOutput")

    FP32 = mybir.dt.float32
    BF = mybir.dt.bfloat16
    SIG = mybir.ActivationFunctionType.Sigmoid
    TANH = mybir.ActivationFunctionType.Tanh

    with tile.TileContext(nc) as tc:
        with tc.tile_pool(name="const", bufs=1) as cpool:
          with tc.tile_pool(name="gbuf", bufs=2) as gpool, \
               tc.tile_pool(name="work", bufs=2) as wpool, \
               tc.tile_pool(name="zps", bufs=2, space="PSUM") as zpool:

            wf = cpool.tile([KDIM, 128], BF)
            nc.sync.dma_start(out=wf[:, :], in_=wf_d.ap())
            # bw weights live at partition base 64: walrus requires matmul
            # fmap and weight to share the same SB start partition, and the
            # bw rhs is G[64:128]. Host pads to [128, 128] (top half zeros)
            # so the DMA itself writes at partition base 0.
            wb_t = cpool.tile([128, 128], BF)
            nc.sync.dma_start(out=wb_t[:, :], in_=wb_d.ap())
            wb = wb_t[64:128, :]
            bv = cpool.tile([128, 4], FP32)
            nc.sync.dma_start(out=bv[:, :], in_=bv_d.ap())
            idx_sb = cpool.tile([128, ntiles * NT * T // 16], mybir.dt.int16)
            nc.sync.dma_start(out=idx_sb[:, :], in_=idx_d.ap())
            state = cpool.tile([64, nseq], BF)

            IDXW = NT * T // 16      # idx columns per tile

            def gather_tile(j):
                g = gpool.tile([128, 1, T * NT], BF, tag=f"g{j % 2}")
                nc.gpsimd.dma_gather(
                    out_ap=g[:, :, :],
                    in_ap=table_d.ap(),
                    idxs_ap=idx_sb[:, j * IDXW:(j + 1) * IDXW],
                    num_idxs=T * NT,
                    num_idxs_reg=T * NT,
                    elem_size=128,
                    transpose=True,
                    single_packet=False,
                )
                return g

            for pair in range(npairs):
                if mode == "empty":
                    break
                if mode == "compute":
                    ga = gpool.tile([128, 1, T * NT], BF, tag="g0",
                                    name=f"ga{pair}")
                    gb = gpool.tile([128, 1, T * NT], BF, tag="g1",
                                    name=f"gb{pair}")
                else:
                    ga = gather_tile(2 * pair)
                    gb = gather_tile(2 * pair + 1)
                gs = [ga, gb]
                if mode == "gather":
                    # consume G so nothing gets dead-code-eliminated
                    for gi2, g_t in enumerate(gs):
                        col0 = (2 * pair + gi2) * NT
                        nc.vector.tensor_copy(
                            state[0:32, col0:col0 + NT],
                            g_t[32:64, 0, (T - 1) * NT:T * NT])
                        nc.vector.tensor_copy(
                            state[32:64, col0:col0 + NT],
                            g_t[64:96, 0, (T - 1) * NT:T * NT])
                    continue
                c_all = wpool.tile([128, NT], FP32, tag="c")

                def sl(off):
                    if isinstance(off, int):
                        return slice(off, off + NT)
                    return bass.ds(off, NT)

                def mm_gate(zt_g, gi, fw_off, bw_off):
                    # 4 streams stacked on PSUM partition quarters
                    for s in range(4):
                        g_t = gs[s // 2]
                        bw = s % 2
                        lo = EMB1 if bw else HFW0
                        w_s = wb if bw else wf
                        rhs = g_t[lo:lo + KDIM, 0, sl(bw_off if bw else fw_off)]
                        nc.tensor.matmul(
                            zt_g[32 * s:32 * s + 32, :],
                            w_s[:, 32 * gi:32 * gi + 32], rhs,
                            start=True, stop=True,
                            tile_position=(64 if bw else 0, 32 * s))

                def lstm_step(fw_off, bw_off, first, last, key):
                    # one PSUM bank per gate, stream s on partition quarter s.
                    # gate issue order g,i,f,o lets each ACT start after only
                    # 4 matmuls and keeps the c-chain off the critical path.
                    zt = [zpool.tile([128, NT], FP32, tag=f"z{gi}",
                                     name=f"z{gi}_{pair}_{key}")
                          for gi in range(4)]
                    mm_gate(zt[2][:, :], 2, fw_off, bw_off)
                    g_all = wpool.tile([128, NT], BF, tag="gall")
                    nc.scalar.activation(g_all[:, :], zt[2][:, :], TANH,
                                         bias=bv[:, 2:3])
                    mm_gate(zt[0][:, :], 0, fw_off, bw_off)
                    ui = wpool.tile([128, NT], BF, tag="ui")
                    nc.scalar.activation(ui[:, :], zt[0][:, :], SIG,
                                         bias=bv[:, 0:1])
                    mm_gate(zt[1][:, :], 1, fw_off, bw_off)
                    uf = wpool.tile([128, NT], BF, tag="uf")
                    nc.scalar.activation(uf[:, :], zt[1][:, :], SIG,
                                         bias=bv[:, 1:2])
                    mm_gate(zt[3][:, :], 3, fw_off, bw_off)
                    uo = wpool.tile([128, NT], BF, tag="uo")
                    nc.scalar.activation(uo[:, :], zt[3][:, :], SIG,
                                         bias=bv[:, 3:4])

                    if first:
                        # c = i * g  (h-slots and previous c are zero)
                        nc.vector.tensor_mul(c_all[:, :], ui[:, :],
                                             g_all[:, :])
                    else:
                        t1 = wpool.tile([128, NT], BF, tag="t1")
                        nc.vector.tensor_mul(t1[:, :], ui[:, :], g_all[:, :])
                        t2 = wpool.tile([128, NT], FP32, tag="t2")
                        nc.vector.tensor_mul(t2[:, :], uf[:, :], c_all[:, :])
                        nc.vector.tensor_add(c_all[:, :], t1[:, :], t2[:, :])

                    tc_t = wpool.tile([128, NT], BF, tag="tc")
                    nc.scalar.activation(tc_t[:, :], c_all[:, :], TANH)

                    # h = o * tanh(c), written straight into the next step's
                    # h-slot of G (or the state tile at the last step)
                    for s in range(4):
                        g_t = gs[s // 2]
                        bw = s % 2
                        if last:
                            col0 = (2 * pair + s // 2) * NT
                            dst = state[32 * bw:32 * bw + 32,
                                        col0:col0 + NT]
                        else:
                            h0 = HBW0 if bw else HFW0
                            hoff = (bw_off - NT) if bw else (fw_off + NT)
                            dst = g_t[h0:h0 + 32, 0, sl(hoff)]
                        nc.vector.tensor_mul(dst,
                                             uo[32 * s:32 * s + 32, :],
                                             tc_t[32 * s:32 * s + 32, :])

                if hw_loop:
                    lstm_step(0, (T - 1) * NT, True, False, "t0")
                    with tc.For_i(NT, (T - 1) * NT, NT,
                                  name=f"tau{pair}") as iv:
                        lstm_step(iv, (T - 1) * NT - iv, False, False, "dyn")
                    lstm_step((T - 1) * NT, 0, False, True, "tZ")
                else:
                    for tau in range(T):
                        lstm_step(tau * NT, (T - 1 - tau) * NT,
                                  tau == 0, tau == T - 1, tau)

          # ---- dense head: out[c, b] = tanh(sum_p Wd_p.T @ state_p + bd)
          if mode == "empty":
            # overhead-floor variant: touch outputs without the LSTM loop
            nc.sync.dma_start(out=out_d.ap()[0:64, 0:4], in_=bv[0:64, 0:4])
          else:
            wd = cpool.tile([64, 4096], BF)
            nc.sync.dma_start(out=wd[:, :], in_=wd_d.ap())
            bd = cpool.tile([64, 1], FP32)
            nc.sync.dma_start(out=bd[:, :], in_=bd_d.ap())

            with tc.tile_pool(name="head", bufs=1, space="PSUM") as hpool:
                st_r = state[:, :].rearrange("p (b q) -> p q b", q=P)
                hp = hpool.tile([64, nbatch], FP32)
                for p in range(P):
                    nc.tensor.matmul(hp[:, :], wd[:, 64 * p:64 * p + 64],
                                     st_r[:, p:p + 1, :],
                                     start=(p == 0), stop=(p == P - 1))
                out_sb = cpool.tile([64, nbatch], FP32)
                nc.scalar.activation(out_sb[:, :], hp[:, :], TANH,
                                     bias=bd[:, :])
                nc.sync.dma_start(out=out_d.ap(), in_=out_sb[:, :])

    nc.compile()
    return nc


def build_kernel_pairloop(nseq=NSEQ):
    """Hardware-loop variant: one For_i over pairs of seq-tiles, LSTM time
    loop unrolled inside the body.  All register-offset APs are partition-
    base-0 (base!=0 + register offset is broken in lowering): the final
    hidden states go to separate [32, nseq] fw/bw tiles, and per-pair index
    slices are DMA-staged from DRAM at dynamic offsets."""
    from concourse.expressions import smin

    ntiles = nseq // NT
    npairs = ntiles // 2
    nbatch = nseq // P
    IDXW = NT * T // 16
    PCOLS = 2 * IDXW              # idx cols per pair

    nc = bacc.Bacc("TRN2", target_bir_lowering=False, debug=False,
                   enable_asserts=False, num_devices=N_CORES)

    table_d = nc.dram_tensor("table", [VOCP, 128], mybir.dt.bfloat16,
                             kind="ExternalInput")
    idx_d = nc.dram_tensor("idx", [128, ntiles * IDXW], mybir.dt.int16,
                           kind="ExternalInput")
    wf_d = nc.dram_tensor("wf", [KDIM, 128], mybir.dt.bfloat16,
                          kind="ExternalInput")
    wb_d = nc.dram_tensor("wb", [128, 128], mybir.dt.bfloat16,
                          kind="ExternalInput")
    bv_d = nc.dram_tensor("bv", [128, 4], mybir.dt.float32,
                          kind="ExternalInput")
    wd_d = nc.dram_tensor("wd", [64, 4096], mybir.dt.bfloat16,
                          kind="ExternalInput")
    bd_d = nc.dram_tensor("bd", [64, 1], mybir.dt.float32,
                          kind="ExternalInput")
    out_d = nc.dram_tensor("out", [64, nbatch], mybir.dt.float32,
                           kind="ExternalOutput")

    FP32 = mybir.dt.float32
    BF = mybir.dt.bfloat16
    SIG = mybir.ActivationFunctionType.Sigmoid
    TANH = mybir.ActivationFunctionType.Tanh

    with tile.TileContext(nc) as tc:
      with tc.tile_pool(name="const", bufs=1) as cpool:
        wf = cpool.tile([KDIM, 128], BF)
        nc.sync.dma_start(out=wf[:, :], in_=wf_d.ap())
        wb_t = cpool.tile([128, 128], BF)
        nc.sync.dma_start(out=wb_t[:, :], in_=wb_d.ap())
        wb = wb_t[64:128, :]
        bv = cpool.tile([128, 4], FP32)
        nc.sync.dma_start(out=bv[:, :], in_=bv_d.ap())
        st_fw = cpool.tile([32, nseq], BF)
        st_bw = cpool.tile([32, nseq], BF)

        with tc.tile_pool(name="gbuf", bufs=1) as gpool, \
             tc.tile_pool(name="ibuf", bufs=2) as ipool, \
             tc.tile_pool(name="work", bufs=2) as wpool, \
             tc.tile_pool(name="zps", bufs=2, space="PSUM") as zpool:

            # two static double-buffer sets; For_i bodies can't rotate pools
            g_set = {k: [gpool.tile([128, 1, T * NT], BF, name=f"g{k}{j}")
                         for j in range(2)] for k in "AB"}

            def load_and_gather(k, idx_off, key):
                idx_sb = ipool.tile([128, PCOLS], mybir.dt.int16,
                                    tag=f"i{k}", name=f"idx{k}_{key}")
                nc.sync.dma_start(out=idx_sb[:, :],
                                  in_=idx_d.ap()[:, bass.ds(idx_off, PCOLS)])
                for j in range(2):
                    nc.gpsimd.dma_gather(
                        out_ap=g_set[k][j][:, :, :],
                        in_ap=table_d.ap(),
                        idxs_ap=idx_sb[:, j * IDXW:(j + 1) * IDXW],
                        num_idxs=T * NT,
                        num_idxs_reg=T * NT,
                        elem_size=128,
                        transpose=True,
                        single_packet=False,
                    )

            def lstm_pair(k, st_off, key):
                gs = g_set[k]
                c_all = wpool.tile([128, NT], FP32, tag=f"c{k}")

                def mm_gate(zt_g, gi, tau):
                    for s in range(4):
                        g_t = gs[s // 2]
                        bw = s % 2
                        blk = (T - 1 - tau) if bw else tau
                        lo = EMB1 if bw else HFW0
                        w_s = wb if bw else wf
                        rhs = g_t[lo:lo + KDIM, 0, blk * NT:(blk + 1) * NT]
                        nc.tensor.matmul(
                            zt_g[32 * s:32 * s + 32, :],
                            w_s[:, 32 * gi:32 * gi + 32], rhs,
                            start=True, stop=True,
                            tile_position=(64 if bw else 0, 32 * s))

                for tau in range(T):
                    zt = [zpool.tile([128, NT], FP32, tag=f"z{gi}",
                                     name=f"z{gi}_{key}_{tau}")
                          for gi in range(4)]
                    mm_gate(zt[2][:, :], 2, tau)
                    g_all = wpool.tile([128, NT], BF, tag="gall")
                    nc.scalar.activation(g_all[:, :], zt[2][:, :], TANH,
                                         bias=bv[:, 2:3])
                    mm_gate(zt[0][:, :], 0, tau)
                    ui = wpool.tile([128, NT], BF, tag="ui")
                    nc.scalar.activation(ui[:, :], zt[0][:, :], SIG,
                                         bias=bv[:, 0:1])
                    mm_gate(zt[1][:, :], 1, tau)
                    uf = wpool.tile([128, NT], BF, tag="uf")
                    nc.scalar.activation(uf[:, :], zt[1][:, :], SIG,
                                         bias=bv[:, 1:2])
                    mm_gate(zt[3][:, :], 3, tau)
                    uo = wpool.tile([128, NT], BF, tag="uo")
                    nc.scalar.activation(uo[:, :], zt[3][:, :], SIG,
                                         bias=bv[:, 3:4])

                    if tau == 0:
                        nc.vector.tensor_mul(c_all[:, :], ui[:, :],
                                             g_all[:, :])
                    else:
                        t1 = wpool.tile([128, NT], BF, tag="t1")
                        nc.vector.tensor_mul(t1[:, :], ui[:, :], g_all[:, :])
                        t2 = wpool.tile([128, NT], FP32, tag="t2")
                        nc.vector.tensor_mul(t2[:, :], uf[:, :], c_all[:, :])
                        nc.vector.tensor_add(c_all[:, :], t1[:, :],
                                             t2[:, :])

                    tc_t = wpool.tile([128, NT], BF, tag="tc")
                    nc.scalar.activation(tc_t[:, :], c_all[:, :], TANH)

                    for s in range(4):
                        g_t = gs[s // 2]
                        bw = s % 2
                        if tau == T - 1:
                            st = st_bw if bw else st_fw
                            dst = st[0:32,
                                     bass.ds(st_off + (s // 2) * NT, NT)]
                        else:
                            nblk = (T - 2 - tau) if bw else (tau + 1)
                            h0 = HBW0 if bw else HFW0
                            dst = g_t[h0:h0 + 32, 0,
                                      nblk * NT:(nblk + 1) * NT]
                        nc.vector.tensor_mul(dst,
                                             uo[32 * s:32 * s + 32, :],
                                             tc_t[32 * s:32 * s + 32, :])

            load_and_gather("A", 0, "p0")
            load_and_gather("B", PCOLS, "p1")
            LAST_A = (npairs - 2) * PCOLS
            LAST_B = (npairs - 1) * PCOLS
            with tc.For_i(0, npairs * PCOLS, 2 * PCOLS,
                          name="pairs") as iv:
                lstm_pair("A", iv, "A")
                load_and_gather("A", smin(iv + 2 * PCOLS, LAST_A), "nA")
                lstm_pair("B", iv + PCOLS, "B")
                load_and_gather("B", smin(iv + 3 * PCOLS, LAST_B), "nB")

        # ---- dense head: split fw/bw contraction, K=32 each
        wd_fw = cpool.tile([32, 4096], BF)
        nc.sync.dma_start(out=wd_fw[:, :], in_=wd_d.ap()[0:32, :])
        wd_bw = cpool.tile([32, 4096], BF)
        nc.sync.dma_start(out=wd_bw[:, :], in_=wd_d.ap()[32:64, :])
        bd = cpool.tile([64, 1], FP32)
        nc.sync.dma_start(out=bd[:, :], in_=bd_d.ap())

        with tc.tile_pool(name="head", bufs=1, space="PSUM") as hpool:
            fw_r = st_fw[:, :].rearrange("p (b q) -> p q b", q=P)
            bw_r = st_bw[:, :].rearrange("p (b q) -> p q b", q=P)
            hp = hpool.tile([64, nbatch], FP32)
            for p in range(P):
                nc.tensor.matmul(hp[:, :], wd_fw[:, 64 * p:64 * p + 64],
                                 fw_r[:, p:p + 1, :],
                                 start=(p == 0), stop=False,
                                 tile_position=(0, 0))
            for p in range(P):
                nc.tensor.matmul(hp[:, :], wd_bw[:, 64 * p:64 * p + 64],
                                 bw_r[:, p:p + 1, :],
                                 start=False, stop=(p == P - 1),
                                 tile_position=(0, 0))
            out_sb = cpool.tile([64, nbatch], FP32)
            nc.scalar.activation(out_sb[:, :], hp[:, :], TANH,
                                 bias=bd[:, :])
            nc.sync.dma_start(out=out_d.ap(), in_=out_sb[:, :])

    nc.compile()
    return nc


# ---------------------------------------------------------------------------
# host-side packing
# ---------------------------------------------------------------------------

def pack_table(embed_table):
    tbl = np.zeros((VOCP, 128), np.float32)
    tbl[:VOC, EMB0:EMB0 + 32] = _f32(embed_table)
    tbl[:VOC, EMB1:EMB1 + 32] = _f32(embed_table)
    return tbl.astype(BF16)


def pack_idx(x_core, nseq=NSEQ):
    """x_core: [nseq, T] int32 -> wrapped int16 [128, ntiles * T*NT/16]."""
    ntiles = nseq // NT
    cols = []
    for j in range(ntiles):
        u = x_core[j * NT:(j + 1) * NT, :].T.reshape(-1)     # t-major [T*NT]
        w = u.reshape(-1, 16).T                               # [16, T*NT/16]
        cols.append(np.tile(w, (8, 1)))
    return np.concatenate(cols, axis=1).astype(np.int16)


def pack_weights(Wk, Wr, b):
    Wk, Wr, b = _f32(Wk), _f32(Wr), _f32(b)
    wf = np.concatenate([Wr, Wk], 0)                          # [64, 128]
    wb = np.concatenate([np.zeros((64, 128), np.float32), Wk, Wr], 0)
    bv = np.tile(b.reshape(4, 32), (1, 4)).reshape(4, 128).T  # [128, 4]
    return wf.astype(BF16), wb.astype(BF16), np.ascontiguousarray(bv, np.float32)


def pack_wd(Wd):
    w = _f32(Wd).reshape(P, 64, 64).transpose(1, 0, 2).reshape(64, 4096)
    return w.astype(BF16)


# ---------------------------------------------------------------------------
# host reference bits for the zero-token fixup
# ---------------------------------------------------------------------------

def _np_lstm_last_h(emb, mask, Wk, Wr, b):
    n = emb.shape[0]
    h = np.zeros((n, H), np.float32)
    c = np.zeros((n, H), np.float32)
    for t in range(emb.shape[1]):
        z = emb[:, t, :] @ Wk + h @ Wr + b
        i = 1.0 / (1.0 + np.exp(-z[:, 0:32]))
        f = 1.0 / (1.0 + np.exp(-z[:, 32:64]))
        g = np.tanh(z[:, 64:96])
        o = 1.0 / (1.0 + np.exp(-z[:, 96:128]))
        c_new = f * c + i * g
        h_new = o * np.tanh(c_new)
        m = mask[:, t][:, None]
        h = np.where(m, h_new, h)
        c = np.where(m, c_new, c)
    return h


def _host_fixup(out, x_flat, embed_table, Wk, Wr, b, Wd, bd):
    """Recompute batch rows whose sequences contain a zero token.

    The device ignores mask_zero (tokens are zero w.p. 1e-4); affected rows
    (~0.1% of the batch) are recomputed exactly on the host, including every
    path of each affected row, so no device state output is needed."""
    mask = x_flat != 0
    bad_seq = np.nonzero(~mask.all(axis=1))[0]
    if bad_seq.size == 0:
        return out
    bad_rows = np.unique(bad_seq // P)
    seqs = (bad_rows[:, None] * P + np.arange(P)[None, :]).reshape(-1)
    emb = _f32(embed_table)[x_flat[seqs]]
    m = mask[seqs]
    h_fw = _np_lstm_last_h(emb, m, _f32(Wk), _f32(Wr), _f32(b))
    h_bw = _np_lstm_last_h(emb[:, ::-1, :], m[:, ::-1],
                           _f32(Wk), _f32(Wr), _f32(b))
    st = np.concatenate([h_fw, h_bw], axis=1)
    st = st.reshape(bad_rows.size, P * 64)
    out[bad_rows] = np.tanh(st @ _f32(Wd) + _f32(bd))
    return out


# ---------------------------------------------------------------------------
# entry point
# ---------------------------------------------------------------------------

_NC_CACHE = {}


def _get_nc(mode="full"):
    hw_loop = os.environ.get("K_HW_LOOP", "0") == "1"
    pair_loop = os.environ.get("K_PAIR_LOOP", "0") == "1"
    key = "nc" + mode + ("hw" if hw_loop else "") + ("pl" if pair_loop else "")
    if key not in _NC_CACHE:
        if pair_loop and mode == "full":
            _NC_CACHE[key] = build_kernel_pairloop()
        else:
            _NC_CACHE[key] = build_kernel(mode=mode, hw_loop=hw_loop)
    return _NC_CACHE[key]


def run_device(inputs, trace=False):
    x = np.asarray(inputs["x"])
    table = pack_table(inputs["embed_table"])
    wf, wb, bv = pack_weights(inputs["Wk"], inputs["Wr"], inputs["b"])
    wd = pack_wd(inputs["Wd"])
    bd = _f32(inputs["bd"]).reshape(64, 1)

    x_flat = x.reshape(B * P, L)
    in_maps = []
    for k in range(N_CORES):
        x_core = x_flat[k * NSEQ:(k + 1) * NSEQ]
        in_maps.append({
            "table": table,
            "idx": pack_idx(x_core),
            "wf": wf,
            "wb": wb,
            "bv": bv,
            "wd": wd,
            "bd": bd,
        })

    nc = _get_nc()
    res = bass_utils.run_bass_kernel_spmd(
        nc, in_maps, core_ids=list(range(N_CORES)), trace=trace)

    out = np.empty((B, 64), np.float32)
    for k in range(N_CORES):
        out[k * B_LOC:(k + 1) * B_LOC] = res.results[k]["out"].T
    return out, res


def kernel(x, embed_table, Wk, Wr, b, Wd, bd):
    inputs = dict(x=x, embed_table=embed_table, Wk=Wk, Wr=Wr, b=b,
                  Wd=Wd, bd=bd)
    out, _ = run_device(inputs)
    out = _host_fixup(out, np.asarray(x).reshape(B * P, L),
                      embed_table, Wk, Wr, b, Wd, bd)
    return out



# revision 19
# speedup vs baseline: 3.2768x; 1.4629x over previous
"""CardEncoder Trainium2 kernel.

Model (per sequence of L=16 tokens): embed(32) -> bidirectional LSTM(32) ->
concat final states -> per-batch dense (4096 -> 64) -> tanh.

Strategy (pure data parallel, 8 cores, batch-sharded):
  * Host packs a gather table [10112, 128] bf16 per vocab row:
      [ h_fw slot (zeros 0:32) | emb (32:64) | emb copy (64:96) |
        h_bw slot (zeros 96:128) ]
  * Device gathers rows with dma_gather(transpose=True) -> feature-major
    tiles G[128, T*NT] (columns = (t, seq)); the zero h-slots are
    overwritten in-place with the running hidden state so each step is one
    K=64 matmul per (gate, stream), biases fused into the activations.
  * 4 streams (2 seq-tiles x {fw, bw}) stacked on PSUM partition quarters
    via tile_position col-groups, so all elementwise work runs on full
    [128, NT] tiles.  Gate issue order g,i,f,o keeps the c-chain off the
    matmul critical path; h = o*tanh(c) is written by per-stream DVE muls
    straight into the next step's h-slot of G (no separate copies).
  * Default build (build_kernel_pairloop): one For_i hardware loop over
    seq-tile pairs with the 16 LSTM steps unrolled in the body (~2.1k
    instructions vs ~13.3k unrolled).  Dynamic (register-offset) access
    patterns are kept at partition base 0 only — base!=0 + register offset
    is mis-lowered (verified against CoreSim and hardware) — hence final
    states go to separate [32, nseq] fw/bw tiles and per-pair index slices
    are DMA-staged from DRAM.  Indices ship as a [32, cols] wrap,
    replicated to 128 partitions on device by two DVE copies.
  * Dense head on device (K=32 matmuls accumulating fw then bw parts);
    host transposes [64, B] -> [B, 64].
  * mask_zero=True handling: token==0 steps must leave (h, c) unchanged.
    Zero tokens occur w.p. 1e-4; the device ignores masking and the host
    recomputes the ~0.1% affected batch rows exactly from x.
"""

import os
import numpy as np
import ml_dtypes

os.environ.setdefault("JAX_PLATFORMS", "cpu")

import concourse.bass as bass
import concourse.bacc as bacc
import concourse.mybir as mybir
import concourse.tile as tile
from concourse import bass_utils

BF16 = ml_dtypes.bfloat16

B, P, L = 2048, 64, 16
H = 32                      # LSTM units per direction
VOC = 10000
VOCP = 10112                # padded to 79 * 128
N_CORES = 8
B_LOC = B // N_CORES        # 256 batches per core
NSEQ = B_LOC * P            # 16384 sequences per core
NT = 512                    # sequences per tile
T = L

# G tile partition layout (SBUF APs must start at partition 0/32/64/96):
#   [ h_fw slot 0:32 | emb 32:64 | emb copy 64:96 | h_bw slot 96:128 ]
# fw rhs = rows 0:64 [h_fw, emb];  bw rhs = rows 64:128 [emb, h_bw].
HFW0 = 0
EMB0 = 32
EMB1 = 64
HBW0 = 96
KDIM = 64                   # matmul contraction size


def _f32(x):
    return np.asarray(x, np.float32)


# ---------------------------------------------------------------------------
# device kernel
# ---------------------------------------------------------------------------

def build_kernel(nseq=NSEQ, mode="full", hw_loop=False):
    ntiles = nseq // NT
    npairs = ntiles // 2
    nbatch = nseq // P

    nc = bacc.Bacc("TRN2", target_bir_lowering=False, debug=False,
                   enable_asserts=False, num_devices=N_CORES)

    table_d = nc.dram_tensor("table", [VOCP, 128], mybir.dt.bfloat16,
                             kind="ExternalInput")
    idx_d = nc.dram_tensor("idx", [128, ntiles * NT * T // 16], mybir.dt.int16,
                           kind="ExternalInput")
    wf_d = nc.dram_tensor("wf", [KDIM, 128], mybir.dt.bfloat16,
                          kind="ExternalInput")
    wb_d = nc.dram_tensor("wb", [128, 128], mybir.dt.bfloat16,
                          kind="ExternalInput")
    bv_d = nc.dram_tensor("bv", [128, 4], mybir.dt.float32,
                          kind="ExternalInput")
    wd_d = nc.dram_tensor("wd", [64, 4096], mybir.dt.bfloat16,
                          kind="ExternalInput")
    bd_d = nc.dram_tensor("bd", [64, 1], mybir.dt.float32,
                          kind="ExternalInput")
    out_d = nc.dram_tensor("out", [64, nbatch], mybir.dt.float32,
                           kind="ExternalOutput")

    FP32 = mybir.dt.float32
    BF = mybir.dt.bfloat16
    SIG = mybir.ActivationFunctionType.Sigmoid
    TANH = mybir.ActivationFunctionType.Tanh

    with tile.TileContext(nc) as tc:
        with tc.tile_pool(name="const", bufs=1) as cpool:
          with tc.tile_pool(name="gbuf", bufs=2) as gpool, \
               tc.tile_pool(name="work", bufs=2) as wpool, \
               tc.tile_pool(name="zps", bufs=2, space="PSUM") as zpool:

            wf = cpool.tile([KDIM, 128], BF)
            nc.sync.dma_start(out=wf[:, :], in_=wf_d.ap())
            # bw weights live at partition base 64: walrus requires matmul
            # fmap and weight to share the same SB start partition, and the
            # bw rhs is G[64:128]. Host pads to [128, 128] (top half zeros)
            # so the DMA itself writes at partition base 0.
            wb_t = cpool.tile([128, 128], BF)
            nc.sync.dma_start(out=wb_t[:, :], in_=wb_d.ap())
            wb = wb_t[64:128, :]
            bv = cpool.tile([128, 4], FP32)
            nc.sync.dma_start(out=bv[:, :], in_=bv_d.ap())
            idx_sb = cpool.tile([128, ntiles * NT * T // 16], mybir.dt.int16)
            nc.sync.dma_start(out=idx_sb[:, :], in_=idx_d.ap())
            state = cpool.tile([64, nseq], BF)

            IDXW = NT * T // 16      # idx columns per tile

            def gather_tile(j):
                g = gpool.tile([128, 1, T * NT], BF, tag=f"g{j % 2}")
                nc.gpsimd.dma_gather(
                    out_ap=g[:, :, :],
                    in_ap=table_d.ap(),
                    idxs_ap=idx_sb[:, j * IDXW:(j + 1) * IDXW],
                    num_idxs=T * NT,
                    num_idxs_reg=T * NT,
                    elem_size=128,
                    transpose=True,
                    single_packet=False,
                )
                return g

            for pair in range(npairs):
                if mode == "empty":
                    break
                if mode == "compute":
                    ga = gpool.tile([128, 1, T * NT], BF, tag="g0",
                                    name=f"ga{pair}")
                    gb = gpool.tile([128, 1, T * NT], BF, tag="g1",
                                    name=f"gb{pair}")
                else:
                    ga = gather_tile(2 * pair)
                    gb = gather_tile(2 * pair + 1)
                gs = [ga, gb]
                if mode == "gather":
                    # consume G so nothing gets dead-code-eliminated
                    for gi2, g_t in enumerate(gs):
                        col0 = (2 * pair + gi2) * NT
                        nc.vector.tensor_copy(
                            state[0:32, col0:col0 + NT],
                            g_t[32:64, 0, (T - 1) * NT:T * NT])
                        nc.vector.tensor_copy(
                            state[32:64, col0:col0 + NT],
                            g_t[64:96, 0, (T - 1) * NT:T * NT])
                    continue
                c_all = wpool.tile([128, NT], FP32, tag="c")

                def sl(off):
                    if isinstance(off, int):
                        return slice(off, off + NT)
                    return bass.ds(off, NT)

                def mm_gate(zt_g, gi, fw_off, bw_off):
                    # 4 streams stacked on PSUM partition quarters
                    for s in range(4):
                        g_t = gs[s // 2]
                        bw = s % 2
                        lo = EMB1 if bw else HFW0
                        w_s = wb if bw else wf
                        rhs = g_t[lo:lo + KDIM, 0, sl(bw_off if bw else fw_off)]
                        nc.tensor.matmul(
                            zt_g[32 * s:32 * s + 32, :],
                            w_s[:, 32 * gi:32 * gi + 32], rhs,
                            start=True, stop=True,
                            tile_position=(64 if bw else 0, 32 * s))

                def lstm_step(fw_off, bw_off, first, last, key):
                    # one PSUM bank per gate, stream s on partition quarter s.
                    # gate issue order g,i,f,o lets each ACT start after only
                    # 4 matmuls and keeps the c-chain off the critical path.
                    zt = [zpool.tile([128, NT], FP32, tag=f"z{gi}",
                                     name=f"z{gi}_{pair}_{key}")
                          for gi in range(4)]
                    mm_gate(zt[2][:, :], 2, fw_off, bw_off)
                    g_all = wpool.tile([128, NT], BF, tag="gall")
                    nc.scalar.activation(g_all[:, :], zt[2][:, :], TANH,
                                         bias=bv[:, 2:3])
                    mm_gate(zt[0][:, :], 0, fw_off, bw_off)
                    ui = wpool.tile([128, NT], BF, tag="ui")
                    nc.scalar.activation(ui[:, :], zt[0][:, :], SIG,
                                         bias=bv[:, 0:1])
                    mm_gate(zt[1][:, :], 1, fw_off, bw_off)
                    uf = wpool.tile([128, NT], BF, tag="uf")
                    nc.scalar.activation(uf[:, :], zt[1][:, :], SIG,
                                         bias=bv[:, 1:2])
                    mm_gate(zt[3][:, :], 3, fw_off, bw_off)
                    uo = wpool.tile([128, NT], BF, tag="uo")
                    nc.scalar.activation(uo[:, :], zt[3][:, :], SIG,
                                         bias=bv[:, 3:4])

                    if first:
                        # c = i * g  (h-slots and previous c are zero)
                        nc.vector.tensor_mul(c_all[:, :], ui[:, :],
                                             g_all[:, :])
                    else:
                        t1 = wpool.tile([128, NT], BF, tag="t1")
                        nc.vector.tensor_mul(t1[:, :], ui[:, :], g_all[:, :])
                        t2 = wpool.tile([128, NT], FP32, tag="t2")
                        nc.vector.tensor_mul(t2[:, :], uf[:, :], c_all[:, :])
                        nc.vector.tensor_add(c_all[:, :], t1[:, :], t2[:, :])

                    tc_t = wpool.tile([128, NT], BF, tag="tc")
                    nc.scalar.activation(tc_t[:, :], c_all[:, :], TANH)

                    # h = o * tanh(c), written straight into the next step's
                    # h-slot of G (or the state tile at the last step)
                    for s in range(4):
                        g_t = gs[s // 2]
                        bw = s % 2
                        if last:
                            col0 = (2 * pair + s // 2) * NT
                            dst = state[32 * bw:32 * bw + 32,
                                        col0:col0 + NT]
                        else:
                            h0 = HBW0 if bw else HFW0
                            hoff = (bw_off - NT) if bw else (fw_off + NT)
                            dst = g_t[h0:h0 + 32, 0, sl(hoff)]
                        nc.vector.tensor_mul(dst,
                                             uo[32 * s:32 * s + 32, :],
                                             tc_t[32 * s:32 * s + 32, :])

                if hw_loop:
                    lstm_step(0, (T - 1) * NT, True, False, "t0")
                    with tc.For_i(NT, (T - 1) * NT, NT,
                                  name=f"tau{pair}") as iv:
                        lstm_step(iv, (T - 1) * NT - iv, False, False, "dyn")
                    lstm_step((T - 1) * NT, 0, False, True, "tZ")
                else:
                    for tau in range(T):
                        lstm_step(tau * NT, (T - 1 - tau) * NT,
                                  tau == 0, tau == T - 1, tau)

          # ---- dense head: out[c, b] = tanh(sum_p Wd_p.T @ state_p + bd)
          if mode == "empty":
            # overhead-floor variant: touch outputs without the LSTM loop
            nc.sync.dma_start(out=out_d.ap()[0:64, 0:4], in_=bv[0:64, 0:4])
          else:
            wd = cpool.tile([64, 4096], BF)
            nc.sync.dma_start(out=wd[:, :], in_=wd_d.ap())
            bd = cpool.tile([64, 1], FP32)
            nc.sync.dma_start(out=bd[:, :], in_=bd_d.ap())

            with tc.tile_pool(name="head", bufs=1, space="PSUM") as hpool:
                st_r = state[:, :].rearrange("p (b q) -> p q b", q=P)
                hp = hpool.tile([64, nbatch], FP32)
                for p in range(P):
                    nc.tensor.matmul(hp[:, :], wd[:, 64 * p:64 * p + 64],
                                     st_r[:, p:p + 1, :],
                                     start=(p == 0), stop=(p == P - 1))
                out_sb = cpool.tile([64, nbatch], FP32)
                nc.scalar.activation(out_sb[:, :], hp[:, :], TANH,
                                     bias=bd[:, :])
                nc.sync.dma_start(out=out_d.ap(), in_=out_sb[:, :])

    nc.compile()
    return nc


def build_kernel_pairloop(nseq=NSEQ):
    """Hardware-loop variant: one For_i over pairs of seq-tiles, LSTM time
    loop unrolled inside the body.  All register-offset APs are partition-
    base-0 (base!=0 + register offset is broken in lowering): the final
    hidden states go to separate [32, nseq] fw/bw tiles, and per-pair index
    slices are DMA-staged from DRAM at dynamic offsets."""
    from concourse.expressions import smin

    ntiles = nseq // NT
    npairs = ntiles // 2
    nbatch = nseq // P
    IDXW = NT * T // 16
    PCOLS = 2 * IDXW              # idx cols per pair

    nc = bacc.Bacc("TRN2", target_bir_lowering=False, debug=False,
                   enable_asserts=False, num_devices=N_CORES)

    table_d = nc.dram_tensor("table", [VOCP, 128], mybir.dt.bfloat16,
                             kind="ExternalInput")
    # indices ship as a 32-partition wrap (16-wrap x2); the gather needs the
    # usual 128-partition replication, done on-device with two DVE copies
    idx_d = nc.dram_tensor("idx", [32, ntiles * IDXW], mybir.dt.int16,
                           kind="ExternalInput")
    wf_d = nc.dram_tensor("wf", [KDIM, 128], mybir.dt.bfloat16,
                          kind="ExternalInput")
    wb_d = nc.dram_tensor("wb", [128, 128], mybir.dt.bfloat16,
                          kind="ExternalInput")
    bv_d = nc.dram_tensor("bv", [128, 4], mybir.dt.float32,
                          kind="ExternalInput")
    wd_d = nc.dram_tensor("wd", [64, 4096], mybir.dt.bfloat16,
                          kind="ExternalInput")
    bd_d = nc.dram_tensor("bd", [64, 1], mybir.dt.float32,
                          kind="ExternalInput")
    out_d = nc.dram_tensor("out", [64, nbatch], mybir.dt.float32,
                           kind="ExternalOutput")

    FP32 = mybir.dt.float32
    BF = mybir.dt.bfloat16
    SIG = mybir.ActivationFunctionType.Sigmoid
    TANH = mybir.ActivationFunctionType.Tanh

    with tile.TileContext(nc) as tc:
      with tc.tile_pool(name="const", bufs=1) as cpool:
        wf = cpool.tile([KDIM, 128], BF)
        nc.sync.dma_start(out=wf[:, :], in_=wf_d.ap())
        wb_t = cpool.tile([128, 128], BF)
        nc.sync.dma_start(out=wb_t[:, :], in_=wb_d.ap())
        wb = wb_t[64:128, :]
        bv = cpool.tile([128, 4], FP32)
        nc.sync.dma_start(out=bv[:, :], in_=bv_d.ap())
        st_fw = cpool.tile([32, nseq], BF)
        st_bw = cpool.tile([32, nseq], BF)

        with tc.tile_pool(name="gbuf", bufs=1) as gpool, \
             tc.tile_pool(name="ibuf", bufs=2) as ipool, \
             tc.tile_pool(name="work", bufs=2) as wpool, \
             tc.tile_pool(name="zps", bufs=2, space="PSUM") as zpool:

            # two static double-buffer sets; For_i bodies can't rotate pools
            g_set = {k: [gpool.tile([128, 1, T * NT], BF, name=f"g{k}{j}")
                         for j in range(2)] for k in "AB"}

            def load_and_gather(k, idx_off, key):
                idx_sb = ipool.tile([128, PCOLS], mybir.dt.int16,
                                    tag=f"i{k}", name=f"idx{k}_{key}")
                nc.sync.dma_start(out=idx_sb[0:32, :],
                                  in_=idx_d.ap()[:, bass.ds(idx_off, PCOLS)])
                nc.vector.tensor_copy(idx_sb[32:64, :], idx_sb[0:32, :])
                nc.vector.tensor_copy(idx_sb[64:128, :], idx_sb[0:64, :])
                for j in range(2):
                    nc.gpsimd.dma_gather(
                        out_ap=g_set[k][j][:, :, :],
                        in_ap=table_d.ap(),
                        idxs_ap=idx_sb[:, j * IDXW:(j + 1) * IDXW],
                        num_idxs=T * NT,
                        num_idxs_reg=T * NT,
                        elem_size=128,
                        transpose=True,
                        single_packet=False,
                    )

            def lstm_pair(k, st_off, key):
                gs = g_set[k]
                c_all = wpool.tile([128, NT], FP32, tag=f"c{k}")

                def mm_gate(zt_g, gi, tau):
                    for s in range(4):
                        g_t = gs[s // 2]
                        bw = s % 2
                        blk = (T - 1 - tau) if bw else tau
                        lo = EMB1 if bw else HFW0
                        w_s = wb if bw else wf
                        rhs = g_t[lo:lo + KDIM, 0, blk * NT:(blk + 1) * NT]
                        nc.tensor.matmul(
                            zt_g[32 * s:32 * s + 32, :],
                            w_s[:, 32 * gi:32 * gi + 32], rhs,
                            start=True, stop=True,
                            tile_position=(64 if bw else 0, 32 * s))

                for tau in range(T):
                    zt = [zpool.tile([128, NT], FP32, tag=f"z{gi}",
                                     name=f"z{gi}_{key}_{tau}")
                          for gi in range(4)]
                    mm_gate(zt[2][:, :], 2, tau)
                    g_all = wpool.tile([128, NT], BF, tag="gall")
                    nc.scalar.activation(g_all[:, :], zt[2][:, :], TANH,
                                         bias=bv[:, 2:3])
                    mm_gate(zt[0][:, :], 0, tau)
                    ui = wpool.tile([128, NT], BF, tag="ui")
                    nc.scalar.activation(ui[:, :], zt[0][:, :], SIG,
                                         bias=bv[:, 0:1])
                    mm_gate(zt[1][:, :], 1, tau)
                    uf = wpool.tile([128, NT], BF, tag="uf")
                    nc.scalar.activation(uf[:, :], zt[1][:, :], SIG,
                                         bias=bv[:, 1:2])
                    mm_gate(zt[3][:, :], 3, tau)
                    uo = wpool.tile([128, NT], BF, tag="uo")
                    nc.scalar.activation(uo[:, :], zt[3][:, :], SIG,
                                         bias=bv[:, 3:4])

                    if tau == 0:
                        nc.vector.tensor_mul(c_all[:, :], ui[:, :],
                                             g_all[:, :])
                    else:
                        t1 = wpool.tile([128, NT], BF, tag="t1")
                        nc.vector.tensor_mul(t1[:, :], ui[:, :], g_all[:, :])
                        t2 = wpool.tile([128, NT], FP32, tag="t2")
                        nc.vector.tensor_mul(t2[:, :], uf[:, :], c_all[:, :])
                        nc.vector.tensor_add(c_all[:, :], t1[:, :],
                                             t2[:, :])

                    tc_t = wpool.tile([128, NT], BF, tag="tc")
                    nc.scalar.activation(tc_t[:, :], c_all[:, :], TANH)

                    for s in range(4):
                        g_t = gs[s // 2]
                        bw = s % 2
                        if tau == T - 1:
                            st = st_bw if bw else st_fw
                            dst = st[0:32,
                                     bass.ds(st_off + (s // 2) * NT, NT)]
                        else:
                            nblk = (T - 2 - tau) if bw else (tau + 1)
                            h0 = HBW0 if bw else HFW0
                            dst = g_t[h0:h0 + 32, 0,
                                      nblk * NT:(nblk + 1) * NT]
                        nc.vector.tensor_mul(dst,
                                             uo[32 * s:32 * s + 32, :],
                                             tc_t[32 * s:32 * s + 32, :])

            load_and_gather("A", 0, "p0")
            load_and_gather("B", PCOLS, "p1")
            LAST_A = (npairs - 2) * PCOLS
            LAST_B = (npairs - 1) * PCOLS
            with tc.For_i(0, npairs * PCOLS, 2 * PCOLS,
                          name="pairs") as iv:
                lstm_pair("A", iv, "A")
                load_and_gather("A", smin(iv + 2 * PCOLS, LAST_A), "nA")
                lstm_pair("B", iv + PCOLS, "B")
                load_and_gather("B", smin(iv + 3 * PCOLS, LAST_B), "nB")

        # ---- dense head: split fw/bw contraction, K=32 each
        wd_fw = cpool.tile([32, 4096], BF)
        nc.sync.dma_start(out=wd_fw[:, :], in_=wd_d.ap()[0:32, :])
        wd_bw = cpool.tile([32, 4096], BF)
        nc.sync.dma_start(out=wd_bw[:, :], in_=wd_d.ap()[32:64, :])
        bd = cpool.tile([64, 1], FP32)
        nc.sync.dma_start(out=bd[:, :], in_=bd_d.ap())

        with tc.tile_pool(name="head", bufs=1, space="PSUM") as hpool:
            fw_r = st_fw[:, :].rearrange("p (b q) -> p q b", q=P)
            bw_r = st_bw[:, :].rearrange("p (b q) -> p q b", q=P)
            hp = hpool.tile([64, nbatch], FP32)
            for p in range(P):
                nc.tensor.matmul(hp[:, :], wd_fw[:, 64 * p:64 * p + 64],
                                 fw_r[:, p:p + 1, :],
                                 start=(p == 0), stop=False,
                                 tile_position=(0, 0))
            for p in range(P):
                nc.tensor.matmul(hp[:, :], wd_bw[:, 64 * p:64 * p + 64],
                                 bw_r[:, p:p + 1, :],
                                 start=False, stop=(p == P - 1),
                                 tile_position=(0, 0))
            out_sb = cpool.tile([64, nbatch], FP32)
            nc.scalar.activation(out_sb[:, :], hp[:, :], TANH,
                                 bias=bd[:, :])
            nc.sync.dma_start(out=out_d.ap(), in_=out_sb[:, :])

    nc.compile()
    return nc


# ---------------------------------------------------------------------------
# host-side packing
# ---------------------------------------------------------------------------

def pack_table(embed_table):
    tbl = np.zeros((VOCP, 128), np.float32)
    tbl[:VOC, EMB0:EMB0 + 32] = _f32(embed_table)
    tbl[:VOC, EMB1:EMB1 + 32] = _f32(embed_table)
    return tbl.astype(BF16)


def pack_idx(x_core, nseq=NSEQ):
    """x_core: [nseq, T] int32 -> wrapped int16 [128, ntiles * T*NT/16]."""
    ntiles = nseq // NT
    cols = []
    for j in range(ntiles):
        u = x_core[j * NT:(j + 1) * NT, :].T.reshape(-1)     # t-major [T*NT]
        w = u.reshape(-1, 16).T                               # [16, T*NT/16]
        cols.append(np.tile(w, (8, 1)))
    return np.concatenate(cols, axis=1).astype(np.int16)


def pack_idx32(x_core, nseq=NSEQ):
    """Like pack_idx but only a 32-partition wrap (replicated on device)."""
    ntiles = nseq // NT
    u = x_core.reshape(ntiles, NT, T).swapaxes(1, 2)          # [nt, T, NT]
    w = u.reshape(ntiles, T * NT // 16, 16)                   # wrap rows
    w = w.transpose(2, 0, 1).reshape(16, -1)                  # [16, nt*cols]
    return np.concatenate([w, w], axis=0).astype(np.int16)


def pack_weights(Wk, Wr, b):
    Wk, Wr, b = _f32(Wk), _f32(Wr), _f32(b)
    wf = np.concatenate([Wr, Wk], 0)                          # [64, 128]
    wb = np.concatenate([np.zeros((64, 128), np.float32), Wk, Wr], 0)
    bv = np.tile(b.reshape(4, 32), (1, 4)).reshape(4, 128).T  # [128, 4]
    return wf.astype(BF16), wb.astype(BF16), np.ascontiguousarray(bv, np.float32)


def pack_wd(Wd):
    w = _f32(Wd).reshape(P, 64, 64).transpose(1, 0, 2).reshape(64, 4096)
    return w.astype(BF16)


# ---------------------------------------------------------------------------
# host reference bits for the zero-token fixup
# ---------------------------------------------------------------------------

def _np_lstm_last_h(emb, mask, Wk, Wr, b):
    n = emb.shape[0]
    h = np.zeros((n, H), np.float32)
    c = np.zeros((n, H), np.float32)
    for t in range(emb.shape[1]):
        z = emb[:, t, :] @ Wk + h @ Wr + b
        i = 1.0 / (1.0 + np.exp(-z[:, 0:32]))
        f = 1.0 / (1.0 + np.exp(-z[:, 32:64]))
        g = np.tanh(z[:, 64:96])
        o = 1.0 / (1.0 + np.exp(-z[:, 96:128]))
        c_new = f * c + i * g
        h_new = o * np.tanh(c_new)
        m = mask[:, t][:, None]
        h = np.where(m, h_new, h)
        c = np.where(m, c_new, c)
    return h


def _host_fixup(out, x_flat, embed_table, Wk, Wr, b, Wd, bd):
    """Recompute batch rows whose sequences contain a zero token.

    The device ignores mask_zero (tokens are zero w.p. 1e-4); affected rows
    (~0.1% of the batch) are recomputed exactly on the host, including every
    path of each affected row, so no device state output is needed."""
    mask = x_flat != 0
    bad_seq = np.nonzero(~mask.all(axis=1))[0]
    if bad_seq.size == 0:
        return out
    bad_rows = np.unique(bad_seq // P)
    seqs = (bad_rows[:, None] * P + np.arange(P)[None, :]).reshape(-1)
    emb = _f32(embed_table)[x_flat[seqs]]
    m = mask[seqs]
    h_fw = _np_lstm_last_h(emb, m, _f32(Wk), _f32(Wr), _f32(b))
    h_bw = _np_lstm_last_h(emb[:, ::-1, :], m[:, ::-1],
                           _f32(Wk), _f32(Wr), _f32(b))
    st = np.concatenate([h_fw, h_bw], axis=1)
    st = st.reshape(bad_rows.size, P * 64)
    out[bad_rows] = np.tanh(st @ _f32(Wd) + _f32(bd))
    return out


# ---------------------------------------------------------------------------
# entry point
# ---------------------------------------------------------------------------

_NC_CACHE = {}


def _use_pair_loop():
    return os.environ.get("K_PAIR_LOOP", "1") == "1"


def _get_nc(mode="full"):
    hw_loop = os.environ.get("K_HW_LOOP", "0") == "1"
    pair_loop = _use_pair_loop()
    key = "nc" + mode + ("hw" if hw_loop else "") + ("pl" if pair_loop else "")
    if key not in _NC_CACHE:
        if pair_loop and mode == "full" and not hw_loop:
            _NC_CACHE[key] = build_kernel_pairloop()
        else:
            _NC_CACHE[key] = build_kernel(mode=mode, hw_loop=hw_loop)
    return _NC_CACHE[key]


def run_device(inputs, trace=False):
    x = np.asarray(inputs["x"])
    table = pack_table(inputs["embed_table"])
    wf, wb, bv = pack_weights(inputs["Wk"], inputs["Wr"], inputs["b"])
    wd = pack_wd(inputs["Wd"])
    bd = _f32(inputs["bd"]).reshape(64, 1)
    pidx = pack_idx32 if _use_pair_loop() else pack_idx

    x_flat = x.reshape(B * P, L)
    in_maps = []
    for k in range(N_CORES):
        x_core = x_flat[k * NSEQ:(k + 1) * NSEQ]
        in_maps.append({
            "table": table,
            "idx": pidx(x_core),
            "wf": wf,
            "wb": wb,
            "bv": bv,
            "wd": wd,
            "bd": bd,
        })

    nc = _get_nc()
    res = bass_utils.run_bass_kernel_spmd(
        nc, in_maps, core_ids=list(range(N_CORES)), trace=trace)

    out = np.empty((B, 64), np.float32)
    for k in range(N_CORES):
        out[k * B_LOC:(k + 1) * B_LOC] = res.results[k]["out"].T
    return out, res


def kernel(x, embed_table, Wk, Wr, b, Wd, bd):
    inputs = dict(x=x, embed_table=embed_table, Wk=Wk, Wr=Wr, b=b,
                  Wd=Wd, bd=bd)
    out, _ = run_device(inputs)
    out = _host_fixup(out, np.asarray(x).reshape(B * P, L),
                      embed_table, Wk, Wr, b, Wd, bd)
    return out



# revision 22
# speedup vs baseline: 3.5194x; 1.0740x over previous
"""CardEncoder Trainium2 kernel.

Model (per sequence of L=16 tokens): embed(32) -> bidirectional LSTM(32) ->
concat final states -> per-batch dense (4096 -> 64) -> tanh.

Strategy (pure data parallel, 8 cores, batch-sharded):
  * Host packs a gather table [10112, 128] bf16 per vocab row:
      [ h_fw slot (zeros 0:32) | emb (32:64) | emb copy (64:96) |
        h_bw slot (zeros 96:128) ]
  * Device gathers rows with dma_gather(transpose=True) -> feature-major
    tiles G[128, T*NT] (columns = (t, seq)); the zero h-slots are
    overwritten in-place with the running hidden state so each step is one
    K=64 matmul per (gate, stream), biases fused into the activations.
  * 4 streams (2 seq-tiles x {fw, bw}) stacked on PSUM partition quarters
    via tile_position col-groups, so all elementwise work runs on full
    [128, NT] tiles.  Gate issue order g,i,f,o keeps the c-chain off the
    matmul critical path; h = o*tanh(c) is written by per-stream DVE muls
    straight into the next step's h-slot of G (no separate copies).
  * Default build (build_kernel_pairloop): one For_i hardware loop over
    seq-tile pairs with the 16 LSTM steps unrolled in the body (~2.1k
    instructions vs ~13.3k unrolled).  Dynamic (register-offset) access
    patterns are kept at partition base 0 only — base!=0 + register offset
    is mis-lowered (verified against CoreSim and hardware) — hence final
    states go to separate [32, nseq] fw/bw tiles and per-pair index slices
    are DMA-staged from DRAM.  Indices ship as a [32, cols] wrap,
    replicated to 128 partitions on device by two DVE copies.
  * Dense head on device (K=32 matmuls accumulating fw then bw parts);
    host transposes [64, B] -> [B, 64].
  * mask_zero=True handling: token==0 steps must leave (h, c) unchanged.
    Zero tokens occur w.p. 1e-4; the device ignores masking and the host
    recomputes the ~0.1% affected batch rows exactly from x.
"""

import os
import numpy as np
import ml_dtypes

os.environ.setdefault("JAX_PLATFORMS", "cpu")

import concourse.bass as bass
import concourse.bacc as bacc
import concourse.mybir as mybir
import concourse.tile as tile
from concourse import bass_utils

BF16 = ml_dtypes.bfloat16

B, P, L = 2048, 64, 16
H = 32                      # LSTM units per direction
VOC = 10000
VOCP = 10112                # padded to 79 * 128
N_CORES = 8
B_LOC = B // N_CORES        # 256 batches per core
NSEQ = B_LOC * P            # 16384 sequences per core
NT = 512                    # sequences per tile
T = L

# G tile partition layout (SBUF APs must start at partition 0/32/64/96):
#   [ h_fw slot 0:32 | emb 32:64 | emb copy 64:96 | h_bw slot 96:128 ]
# fw rhs = rows 0:64 [h_fw, emb];  bw rhs = rows 64:128 [emb, h_bw].
HFW0 = 0
EMB0 = 32
EMB1 = 64
HBW0 = 96
KDIM = 64                   # matmul contraction size


def _f32(x):
    return np.asarray(x, np.float32)


# ---------------------------------------------------------------------------
# device kernel
# ---------------------------------------------------------------------------

def build_kernel(nseq=NSEQ, mode="full", hw_loop=False):
    ntiles = nseq // NT
    npairs = ntiles // 2
    nbatch = nseq // P

    nc = bacc.Bacc("TRN2", target_bir_lowering=False, debug=False,
                   enable_asserts=False, num_devices=N_CORES)

    table_d = nc.dram_tensor("table", [VOCP, 128], mybir.dt.bfloat16,
                             kind="ExternalInput")
    idx_d = nc.dram_tensor("idx", [128, ntiles * NT * T // 16], mybir.dt.int16,
                           kind="ExternalInput")
    wf_d = nc.dram_tensor("wf", [KDIM, 128], mybir.dt.bfloat16,
                          kind="ExternalInput")
    wb_d = nc.dram_tensor("wb", [128, 128], mybir.dt.bfloat16,
                          kind="ExternalInput")
    bv_d = nc.dram_tensor("bv", [128, 4], mybir.dt.float32,
                          kind="ExternalInput")
    wd_d = nc.dram_tensor("wd", [64, 4096], mybir.dt.bfloat16,
                          kind="ExternalInput")
    bd_d = nc.dram_tensor("bd", [64, 1], mybir.dt.float32,
                          kind="ExternalInput")
    out_d = nc.dram_tensor("out", [64, nbatch], mybir.dt.float32,
                           kind="ExternalOutput")

    FP32 = mybir.dt.float32
    BF = mybir.dt.bfloat16
    SIG = mybir.ActivationFunctionType.Sigmoid
    TANH = mybir.ActivationFunctionType.Tanh

    with tile.TileContext(nc) as tc:
        with tc.tile_pool(name="const", bufs=1) as cpool:
          with tc.tile_pool(name="gbuf", bufs=2) as gpool, \
               tc.tile_pool(name="work", bufs=2) as wpool, \
               tc.tile_pool(name="zps", bufs=2, space="PSUM") as zpool:

            wf = cpool.tile([KDIM, 128], BF)
            nc.sync.dma_start(out=wf[:, :], in_=wf_d.ap())
            # bw weights live at partition base 64: walrus requires matmul
            # fmap and weight to share the same SB start partition, and the
            # bw rhs is G[64:128]. Host pads to [128, 128] (top half zeros)
            # so the DMA itself writes at partition base 0.
            wb_t = cpool.tile([128, 128], BF)
            nc.sync.dma_start(out=wb_t[:, :], in_=wb_d.ap())
            wb = wb_t[64:128, :]
            bv = cpool.tile([128, 4], FP32)
            nc.sync.dma_start(out=bv[:, :], in_=bv_d.ap())
            idx_sb = cpool.tile([128, ntiles * NT * T // 16], mybir.dt.int16)
            nc.sync.dma_start(out=idx_sb[:, :], in_=idx_d.ap())
            state = cpool.tile([64, nseq], BF)

            IDXW = NT * T // 16      # idx columns per tile

            def gather_tile(j):
                g = gpool.tile([128, 1, T * NT], BF, tag=f"g{j % 2}")
                nc.gpsimd.dma_gather(
                    out_ap=g[:, :, :],
                    in_ap=table_d.ap(),
                    idxs_ap=idx_sb[:, j * IDXW:(j + 1) * IDXW],
                    num_idxs=T * NT,
                    num_idxs_reg=T * NT,
                    elem_size=128,
                    transpose=True,
                    single_packet=False,
                )
                return g

            for pair in range(npairs):
                if mode == "empty":
                    break
                if mode == "compute":
                    ga = gpool.tile([128, 1, T * NT], BF, tag="g0",
                                    name=f"ga{pair}")
                    gb = gpool.tile([128, 1, T * NT], BF, tag="g1",
                                    name=f"gb{pair}")
                else:
                    ga = gather_tile(2 * pair)
                    gb = gather_tile(2 * pair + 1)
                gs = [ga, gb]
                if mode == "gather":
                    # consume G so nothing gets dead-code-eliminated
                    for gi2, g_t in enumerate(gs):
                        col0 = (2 * pair + gi2) * NT
                        nc.vector.tensor_copy(
                            state[0:32, col0:col0 + NT],
                            g_t[32:64, 0, (T - 1) * NT:T * NT])
                        nc.vector.tensor_copy(
                            state[32:64, col0:col0 + NT],
                            g_t[64:96, 0, (T - 1) * NT:T * NT])
                    continue
                c_all = wpool.tile([128, NT], FP32, tag="c")

                def sl(off):
                    if isinstance(off, int):
                        return slice(off, off + NT)
                    return bass.ds(off, NT)

                def mm_gate(zt_g, gi, fw_off, bw_off):
                    # 4 streams stacked on PSUM partition quarters
                    for s in range(4):
                        g_t = gs[s // 2]
                        bw = s % 2
                        lo = EMB1 if bw else HFW0
                        w_s = wb if bw else wf
                        rhs = g_t[lo:lo + KDIM, 0, sl(bw_off if bw else fw_off)]
                        nc.tensor.matmul(
                            zt_g[32 * s:32 * s + 32, :],
                            w_s[:, 32 * gi:32 * gi + 32], rhs,
                            start=True, stop=True,
                            tile_position=(64 if bw else 0, 32 * s))

                def lstm_step(fw_off, bw_off, first, last, key):
                    # one PSUM bank per gate, stream s on partition quarter s.
                    # gate issue order g,i,f,o lets each ACT start after only
                    # 4 matmuls and keeps the c-chain off the critical path.
                    zt = [zpool.tile([128, NT], FP32, tag=f"z{gi}",
                                     name=f"z{gi}_{pair}_{key}")
                          for gi in range(4)]
                    mm_gate(zt[2][:, :], 2, fw_off, bw_off)
                    g_all = wpool.tile([128, NT], BF, tag="gall")
                    nc.scalar.activation(g_all[:, :], zt[2][:, :], TANH,
                                         bias=bv[:, 2:3])
                    mm_gate(zt[0][:, :], 0, fw_off, bw_off)
                    ui = wpool.tile([128, NT], BF, tag="ui")
                    nc.scalar.activation(ui[:, :], zt[0][:, :], SIG,
                                         bias=bv[:, 0:1])
                    mm_gate(zt[1][:, :], 1, fw_off, bw_off)
                    uf = wpool.tile([128, NT], BF, tag="uf")
                    nc.scalar.activation(uf[:, :], zt[1][:, :], SIG,
                                         bias=bv[:, 1:2])
                    mm_gate(zt[3][:, :], 3, fw_off, bw_off)
                    uo = wpool.tile([128, NT], BF, tag="uo")
                    nc.scalar.activation(uo[:, :], zt[3][:, :], SIG,
                                         bias=bv[:, 3:4])

                    if first:
                        # c = i * g  (h-slots and previous c are zero)
                        nc.vector.tensor_mul(c_all[:, :], ui[:, :],
                                             g_all[:, :])
                    else:
                        t1 = wpool.tile([128, NT], BF, tag="t1")
                        nc.vector.tensor_mul(t1[:, :], ui[:, :], g_all[:, :])
                        t2 = wpool.tile([128, NT], FP32, tag="t2")
                        nc.vector.tensor_mul(t2[:, :], uf[:, :], c_all[:, :])
                        nc.vector.tensor_add(c_all[:, :], t1[:, :], t2[:, :])

                    tc_t = wpool.tile([128, NT], BF, tag="tc")
                    nc.scalar.activation(tc_t[:, :], c_all[:, :], TANH)

                    # h = o * tanh(c), written straight into the next step's
                    # h-slot of G (or the state tile at the last step)
                    for s in range(4):
                        g_t = gs[s // 2]
                        bw = s % 2
                        if last:
                            col0 = (2 * pair + s // 2) * NT
                            dst = state[32 * bw:32 * bw + 32,
                                        col0:col0 + NT]
                        else:
                            h0 = HBW0 if bw else HFW0
                            hoff = (bw_off - NT) if bw else (fw_off + NT)
                            dst = g_t[h0:h0 + 32, 0, sl(hoff)]
                        nc.vector.tensor_mul(dst,
                                             uo[32 * s:32 * s + 32, :],
                                             tc_t[32 * s:32 * s + 32, :])

                if hw_loop:
                    lstm_step(0, (T - 1) * NT, True, False, "t0")
                    with tc.For_i(NT, (T - 1) * NT, NT,
                                  name=f"tau{pair}") as iv:
                        lstm_step(iv, (T - 1) * NT - iv, False, False, "dyn")
                    lstm_step((T - 1) * NT, 0, False, True, "tZ")
                else:
                    for tau in range(T):
                        lstm_step(tau * NT, (T - 1 - tau) * NT,
                                  tau == 0, tau == T - 1, tau)

          # ---- dense head: out[c, b] = tanh(sum_p Wd_p.T @ state_p + bd)
          if mode == "empty":
            # overhead-floor variant: touch outputs without the LSTM loop
            nc.sync.dma_start(out=out_d.ap()[0:64, 0:4], in_=bv[0:64, 0:4])
          else:
            wd = cpool.tile([64, 4096], BF)
            nc.sync.dma_start(out=wd[:, :], in_=wd_d.ap())
            bd = cpool.tile([64, 1], FP32)
            nc.sync.dma_start(out=bd[:, :], in_=bd_d.ap())

            with tc.tile_pool(name="head", bufs=1, space="PSUM") as hpool:
                st_r = state[:, :].rearrange("p (b q) -> p q b", q=P)
                hp = hpool.tile([64, nbatch], FP32)
                for p in range(P):
                    nc.tensor.matmul(hp[:, :], wd[:, 64 * p:64 * p + 64],
                                     st_r[:, p:p + 1, :],
                                     start=(p == 0), stop=(p == P - 1))
                out_sb = cpool.tile([64, nbatch], FP32)
                nc.scalar.activation(out_sb[:, :], hp[:, :], TANH,
                                     bias=bd[:, :])
                nc.sync.dma_start(out=out_d.ap(), in_=out_sb[:, :])

    nc.compile()
    return nc


def build_kernel_pairloop(nseq=NSEQ):
    """Hardware-loop variant: one For_i over pairs of seq-tiles, LSTM time
    loop unrolled inside the body.  All register-offset APs are partition-
    base-0 (base!=0 + register offset is broken in lowering): the final
    hidden states go to separate [32, nseq] fw/bw tiles, and per-pair index
    slices are DMA-staged from DRAM at dynamic offsets."""
    from concourse.expressions import smin

    ntiles = nseq // NT
    npairs = ntiles // 2
    nbatch = nseq // P
    IDXW = NT * T // 16
    PCOLS = 2 * IDXW              # idx cols per pair

    nc = bacc.Bacc("TRN2", target_bir_lowering=False, debug=False,
                   enable_asserts=False, num_devices=N_CORES)

    table_d = nc.dram_tensor("table", [VOCP, 128], mybir.dt.bfloat16,
                             kind="ExternalInput")
    # indices ship as a 32-partition wrap (16-wrap x2); the gather needs the
    # usual 128-partition replication, done on-device with two DVE copies
    idx_d = nc.dram_tensor("idx", [32, ntiles * IDXW], mybir.dt.int16,
                           kind="ExternalInput")
    wf_d = nc.dram_tensor("wf", [KDIM, 128], mybir.dt.bfloat16,
                          kind="ExternalInput")
    wb_d = nc.dram_tensor("wb", [128, 128], mybir.dt.bfloat16,
                          kind="ExternalInput")
    bv_d = nc.dram_tensor("bv", [128, 4], mybir.dt.float32,
                          kind="ExternalInput")
    wd_d = nc.dram_tensor("wd", [64, 4096], mybir.dt.bfloat16,
                          kind="ExternalInput")
    bd_d = nc.dram_tensor("bd", [64, 1], mybir.dt.float32,
                          kind="ExternalInput")
    out_d = nc.dram_tensor("out", [64, nbatch], mybir.dt.float32,
                           kind="ExternalOutput")

    FP32 = mybir.dt.float32
    BF = mybir.dt.bfloat16
    SIG = mybir.ActivationFunctionType.Sigmoid
    TANH = mybir.ActivationFunctionType.Tanh

    with tile.TileContext(nc) as tc:
      with tc.tile_pool(name="const", bufs=1) as cpool:
        wf = cpool.tile([KDIM, 128], BF)
        nc.sync.dma_start(out=wf[:, :], in_=wf_d.ap())
        wb_t = cpool.tile([128, 128], BF)
        nc.sync.dma_start(out=wb_t[:, :], in_=wb_d.ap())
        wb = wb_t[64:128, :]
        bv = cpool.tile([128, 4], FP32)
        nc.sync.dma_start(out=bv[:, :], in_=bv_d.ap())
        st_fw = cpool.tile([32, nseq], BF)
        st_bw = cpool.tile([32, nseq], BF)

        with tc.tile_pool(name="gbuf", bufs=1) as gpool, \
             tc.tile_pool(name="ibuf", bufs=2) as ipool, \
             tc.tile_pool(name="work", bufs=2) as wpool, \
             tc.tile_pool(name="zps", bufs=2, space="PSUM") as zpool:

            # two static double-buffer sets; For_i bodies can't rotate pools
            g_set = {k: [gpool.tile([128, 1, T * NT], BF, name=f"g{k}{j}")
                         for j in range(2)] for k in "AB"}

            def load_and_gather(k, idx_off, key):
                idx_sb = ipool.tile([128, PCOLS], mybir.dt.int16,
                                    tag=f"i{k}", name=f"idx{k}_{key}")
                nc.sync.dma_start(out=idx_sb[0:32, :],
                                  in_=idx_d.ap()[:, bass.ds(idx_off, PCOLS)])
                nc.vector.tensor_copy(idx_sb[32:64, :], idx_sb[0:32, :])
                nc.vector.tensor_copy(idx_sb[64:128, :], idx_sb[0:64, :])
                for j in range(2):
                    nc.gpsimd.dma_gather(
                        out_ap=g_set[k][j][:, :, :],
                        in_ap=table_d.ap(),
                        idxs_ap=idx_sb[:, j * IDXW:(j + 1) * IDXW],
                        num_idxs=T * NT,
                        num_idxs_reg=T * NT,
                        elem_size=128,
                        transpose=True,
                        single_packet=False,
                    )

            def make_step(k, st_off, key):
                """Per-tau LSTM step closure for pair-slot k.  A and B slots
                are interleaved per tau in the loop body so each pair's
                serial MM->ACT->DVE->h-write chain hides behind the other
                pair's matmuls (engine queues are in-order FIFO)."""
                gs = g_set[k]
                c_all = wpool.tile([128, NT], FP32, tag=f"c{k}")

                def mm_gate(zt_g, gi, tau):
                    for s in range(4):
                        g_t = gs[s // 2]
                        bw = s % 2
                        blk = (T - 1 - tau) if bw else tau
                        lo = EMB1 if bw else HFW0
                        w_s = wb if bw else wf
                        rhs = g_t[lo:lo + KDIM, 0, blk * NT:(blk + 1) * NT]
                        nc.tensor.matmul(
                            zt_g[32 * s:32 * s + 32, :],
                            w_s[:, 32 * gi:32 * gi + 32], rhs,
                            start=True, stop=True,
                            tile_position=(64 if bw else 0, 32 * s))

                def step(tau):
                    zt = [zpool.tile([128, NT], FP32, tag=f"z{gi}",
                                     name=f"z{gi}_{key}_{tau}")
                          for gi in range(4)]
                    mm_gate(zt[2][:, :], 2, tau)
                    g_all = wpool.tile([128, NT], BF, tag="gall")
                    nc.scalar.activation(g_all[:, :], zt[2][:, :], TANH,
                                         bias=bv[:, 2:3])
                    mm_gate(zt[0][:, :], 0, tau)
                    ui = wpool.tile([128, NT], BF, tag="ui")
                    nc.scalar.activation(ui[:, :], zt[0][:, :], SIG,
                                         bias=bv[:, 0:1])
                    mm_gate(zt[1][:, :], 1, tau)
                    uf = wpool.tile([128, NT], BF, tag="uf")
                    nc.scalar.activation(uf[:, :], zt[1][:, :], SIG,
                                         bias=bv[:, 1:2])
                    mm_gate(zt[3][:, :], 3, tau)
                    uo = wpool.tile([128, NT], BF, tag="uo")
                    nc.scalar.activation(uo[:, :], zt[3][:, :], SIG,
                                         bias=bv[:, 3:4])

                    if tau == 0:
                        nc.vector.tensor_mul(c_all[:, :], ui[:, :],
                                             g_all[:, :])
                    else:
                        t1 = wpool.tile([128, NT], BF, tag="t1")
                        nc.vector.tensor_mul(t1[:, :], ui[:, :], g_all[:, :])
                        t2 = wpool.tile([128, NT], FP32, tag="t2")
                        nc.vector.tensor_mul(t2[:, :], uf[:, :], c_all[:, :])
                        nc.vector.tensor_add(c_all[:, :], t1[:, :],
                                             t2[:, :])

                    tc_t = wpool.tile([128, NT], BF, tag="tc")
                    nc.scalar.activation(tc_t[:, :], c_all[:, :], TANH)

                    for s in range(4):
                        g_t = gs[s // 2]
                        bw = s % 2
                        if tau == T - 1:
                            st = st_bw if bw else st_fw
                            dst = st[0:32,
                                     bass.ds(st_off + (s // 2) * NT, NT)]
                        else:
                            nblk = (T - 2 - tau) if bw else (tau + 1)
                            h0 = HBW0 if bw else HFW0
                            dst = g_t[h0:h0 + 32, 0,
                                      nblk * NT:(nblk + 1) * NT]
                        nc.vector.tensor_mul(dst,
                                             uo[32 * s:32 * s + 32, :],
                                             tc_t[32 * s:32 * s + 32, :])

                return step

            load_and_gather("A", 0, "p0")
            load_and_gather("B", PCOLS, "p1")
            LAST_A = (npairs - 2) * PCOLS
            LAST_B = (npairs - 1) * PCOLS
            with tc.For_i(0, npairs * PCOLS, 2 * PCOLS,
                          name="pairs") as iv:
                step_a = make_step("A", iv, "A")
                step_b = make_step("B", iv + PCOLS, "B")
                for tau in range(T):
                    step_a(tau)
                    if tau == T - 1:
                        # G_A's last read just issued; refill overlaps B's
                        # tail and the next iteration's early steps
                        load_and_gather("A", smin(iv + 2 * PCOLS, LAST_A),
                                        "nA")
                    step_b(tau)
                load_and_gather("B", smin(iv + 3 * PCOLS, LAST_B), "nB")

        # ---- dense head: split fw/bw contraction, K=32 each
        wd_fw = cpool.tile([32, 4096], BF)
        nc.sync.dma_start(out=wd_fw[:, :], in_=wd_d.ap()[0:32, :])
        wd_bw = cpool.tile([32, 4096], BF)
        nc.sync.dma_start(out=wd_bw[:, :], in_=wd_d.ap()[32:64, :])
        bd = cpool.tile([64, 1], FP32)
        nc.sync.dma_start(out=bd[:, :], in_=bd_d.ap())

        with tc.tile_pool(name="head", bufs=1, space="PSUM") as hpool:
            fw_r = st_fw[:, :].rearrange("p (b q) -> p q b", q=P)
            bw_r = st_bw[:, :].rearrange("p (b q) -> p q b", q=P)
            hp = hpool.tile([64, nbatch], FP32)
            for p in range(P):
                nc.tensor.matmul(hp[:, :], wd_fw[:, 64 * p:64 * p + 64],
                                 fw_r[:, p:p + 1, :],
                                 start=(p == 0), stop=False,
                                 tile_position=(0, 0))
            for p in range(P):
                nc.tensor.matmul(hp[:, :], wd_bw[:, 64 * p:64 * p + 64],
                                 bw_r[:, p:p + 1, :],
                                 start=False, stop=(p == P - 1),
                                 tile_position=(0, 0))
            out_sb = cpool.tile([64, nbatch], FP32)
            nc.scalar.activation(out_sb[:, :], hp[:, :], TANH,
                                 bias=bd[:, :])
            nc.sync.dma_start(out=out_d.ap(), in_=out_sb[:, :])

    nc.compile()
    return nc


# ---------------------------------------------------------------------------
# host-side packing
# ---------------------------------------------------------------------------

def pack_table(embed_table):
    tbl = np.zeros((VOCP, 128), np.float32)
    tbl[:VOC, EMB0:EMB0 + 32] = _f32(embed_table)
    tbl[:VOC, EMB1:EMB1 + 32] = _f32(embed_table)
    return tbl.astype(BF16)


def pack_idx(x_core, nseq=NSEQ):
    """x_core: [nseq, T] int32 -> wrapped int16 [128, ntiles * T*NT/16]."""
    ntiles = nseq // NT
    cols = []
    for j in range(ntiles):
        u = x_core[j * NT:(j + 1) * NT, :].T.reshape(-1)     # t-major [T*NT]
        w = u.reshape(-1, 16).T                               # [16, T*NT/16]
        cols.append(np.tile(w, (8, 1)))
    return np.concatenate(cols, axis=1).astype(np.int16)


def pack_idx32(x_core, nseq=NSEQ):
    """Like pack_idx but only a 32-partition wrap (replicated on device)."""
    ntiles = nseq // NT
    u = x_core.reshape(ntiles, NT, T).swapaxes(1, 2)          # [nt, T, NT]
    w = u.reshape(ntiles, T * NT // 16, 16)                   # wrap rows
    w = w.transpose(2, 0, 1).reshape(16, -1)                  # [16, nt*cols]
    return np.concatenate([w, w], axis=0).astype(np.int16)


def pack_weights(Wk, Wr, b):
    Wk, Wr, b = _f32(Wk), _f32(Wr), _f32(b)
    wf = np.concatenate([Wr, Wk], 0)                          # [64, 128]
    wb = np.concatenate([np.zeros((64, 128), np.float32), Wk, Wr], 0)
    bv = np.tile(b.reshape(4, 32), (1, 4)).reshape(4, 128).T  # [128, 4]
    return wf.astype(BF16), wb.astype(BF16), np.ascontiguousarray(bv, np.float32)


def pack_wd(Wd):
    w = _f32(Wd).reshape(P, 64, 64).transpose(1, 0, 2).reshape(64, 4096)
    return w.astype(BF16)


# ---------------------------------------------------------------------------
# host reference bits for the zero-token fixup
# ---------------------------------------------------------------------------

def _np_lstm_last_h(emb, mask, Wk, Wr, b):
    n = emb.shape[0]
    h = np.zeros((n, H), np.float32)
    c = np.zeros((n, H), np.float32)
    for t in range(emb.shape[1]):
        z = emb[:, t, :] @ Wk + h @ Wr + b
        i = 1.0 / (1.0 + np.exp(-z[:, 0:32]))
        f = 1.0 / (1.0 + np.exp(-z[:, 32:64]))
        g = np.tanh(z[:, 64:96])
        o = 1.0 / (1.0 + np.exp(-z[:, 96:128]))
        c_new = f * c + i * g
        h_new = o * np.tanh(c_new)
        m = mask[:, t][:, None]
        h = np.where(m, h_new, h)
        c = np.where(m, c_new, c)
    return h


def _host_fixup(out, x_flat, embed_table, Wk, Wr, b, Wd, bd):
    """Recompute batch rows whose sequences contain a zero token.

    The device ignores mask_zero (tokens are zero w.p. 1e-4); affected rows
    (~0.1% of the batch) are recomputed exactly on the host, including every
    path of each affected row, so no device state output is needed."""
    mask = x_flat != 0
    bad_seq = np.nonzero(~mask.all(axis=1))[0]
    if bad_seq.size == 0:
        return out
    bad_rows = np.unique(bad_seq // P)
    seqs = (bad_rows[:, None] * P + np.arange(P)[None, :]).reshape(-1)
    emb = _f32(embed_table)[x_flat[seqs]]
    m = mask[seqs]
    h_fw = _np_lstm_last_h(emb, m, _f32(Wk), _f32(Wr), _f32(b))
    h_bw = _np_lstm_last_h(emb[:, ::-1, :], m[:, ::-1],
                           _f32(Wk), _f32(Wr), _f32(b))
    st = np.concatenate([h_fw, h_bw], axis=1)
    st = st.reshape(bad_rows.size, P * 64)
    out[bad_rows] = np.tanh(st @ _f32(Wd) + _f32(bd))
    return out


# ---------------------------------------------------------------------------
# entry point
# ---------------------------------------------------------------------------

_NC_CACHE = {}


def _use_pair_loop():
    return os.environ.get("K_PAIR_LOOP", "1") == "1"


def _get_nc(mode="full"):
    hw_loop = os.environ.get("K_HW_LOOP", "0") == "1"
    pair_loop = _use_pair_loop()
    key = "nc" + mode + ("hw" if hw_loop else "") + ("pl" if pair_loop else "")
    if key not in _NC_CACHE:
        if pair_loop and mode == "full" and not hw_loop:
            _NC_CACHE[key] = build_kernel_pairloop()
        else:
            _NC_CACHE[key] = build_kernel(mode=mode, hw_loop=hw_loop)
    return _NC_CACHE[key]


def run_device(inputs, trace=False):
    x = np.asarray(inputs["x"])
    table = pack_table(inputs["embed_table"])
    wf, wb, bv = pack_weights(inputs["Wk"], inputs["Wr"], inputs["b"])
    wd = pack_wd(inputs["Wd"])
    bd = _f32(inputs["bd"]).reshape(64, 1)
    pidx = pack_idx32 if _use_pair_loop() else pack_idx

    x_flat = x.reshape(B * P, L)
    in_maps = []
    for k in range(N_CORES):
        x_core = x_flat[k * NSEQ:(k + 1) * NSEQ]
        in_maps.append({
            "table": table,
            "idx": pidx(x_core),
            "wf": wf,
            "wb": wb,
            "bv": bv,
            "wd": wd,
            "bd": bd,
        })

    nc = _get_nc()
    res = bass_utils.run_bass_kernel_spmd(
        nc, in_maps, core_ids=list(range(N_CORES)), trace=trace)

    out = np.empty((B, 64), np.float32)
    for k in range(N_CORES):
        out[k * B_LOC:(k + 1) * B_LOC] = res.results[k]["out"].T
    return out, res


def kernel(x, embed_table, Wk, Wr, b, Wd, bd):
    inputs = dict(x=x, embed_table=embed_table, Wk=Wk, Wr=Wr, b=b,
                  Wd=Wd, bd=bd)
    out, _ = run_device(inputs)
    out = _host_fixup(out, np.asarray(x).reshape(B * P, L),
                      embed_table, Wk, Wr, b, Wd, bd)
    return out

